# revision 1
# baseline (speedup 1.0000x reference)
# Multi-head attention (B=2, T=2048, C=768, H=12, D=64) on 8 NeuronCores.
#
# Sharding: core i handles batch b = i // 4 and head group g = i % 4
# (3 heads each).  Each core computes, for its batch slice x_b [T, C]:
#   q/k/v = x_b @ w{q,k,v}[:, g*192:(g+1)*192]          (3 local heads)
#   rope + rmsnorm on q, k; full (non-causal) softmax attention per head
#   partial = attn_out @ wproj[g*192:(g+1)*192, :]       -> [T, C]
# The host sums the 4 partials per batch to form the full output.
#
# Single-core layout strategy (matmul inputs bf16, fp32 accumulation):
#   xT   [128, 6, 2048]  x_b transposed (feature on partition), via PE
#   q/k = xT_chunk.T @ [wq|wk] (fused N=384), v separate (N=192)
#   rope/rmsnorm in [token, head*64] layout (fp32, DVE + ACT Ln/Exp)
#   qT01/kT01 [128, 2048]: rows 0:64 = head0 (d-major), rows 64:128 = head1
#   qT22/kT22 [128, 2048]: head2 duplicated in both halves
#     -> K=64 score matmuls packed two-per-window via PE row tiling
#   scores s^T [tk,tq] = kT.T @ qT ; p = exp(s/8) (ACT, bf16 out, fused
#     across the two heads' psum banks); AV: yplus += [v | 1].T @ p
#   softmax denom = ones-row of yplus; 1/denom on DVE reciprocal,
#   gpsimd partition-broadcast, DVE multiply -> yT (d on partition)
#   proj: out = yT.T @ wp slices, accumulated over the 192 local dims.

import numpy as np
from contextlib import ExitStack

import concourse.hw_specs as _hw_specs
from concourse import mybir

AF = mybir.ActivationFunctionType
ALU = mybir.AluOpType

# Keep Exp/Ln in exactly one ACT table set so bacc's greedy set selection
# never bounces between table sets (each bounce is a ~1.3us table DMA).
if not getattr(_hw_specs, "_mha_act_patch", False):
    _orig_gat = _hw_specs.get_activation_tables

    def _gat_one_exp_ln_set(arch):
        tabs = _orig_gat(arch)
        for name, s in tabs.items():
            if name != "natural_log_exp_and_others":
                s.discard(AF.Exp)
                s.discard(AF.Ln)
        return tabs

    _hw_specs.get_activation_tables = _gat_one_exp_ln_set
    _hw_specs._mha_act_patch = True

import concourse.bass as bass          # noqa: E402
import concourse.tile as tile          # noqa: E402
from concourse import bacc             # noqa: E402
bacc.get_activation_tables = _hw_specs.get_activation_tables
from concourse.bass import ts          # noqa: E402
from concourse.bass_utils import run_bass_kernel_spmd  # noqa: E402
from concourse.masks import make_identity              # noqa: E402

F32 = mybir.dt.float32
BF16 = mybir.dt.bfloat16

T = 2048
C = 768
HL = 3          # heads per core
D = 64
NG = HL * D     # 192, per-core qkv width
NT = T // 128   # 16 token tiles
KC = C // 128   # 6 contraction chunks
TQB = 512       # tq block
NTQ = T // TQB  # 4


def build_kernel(tc, ctx, x, cos, sin, wq, wk, wv, wp, y):
    nc = tc.nc

    const = ctx.enter_context(tc.tile_pool(name="const", bufs=1))
    big = ctx.enter_context(tc.tile_pool(name="big", bufs=1))

    identF = const.tile([128, 128], F32, tag="identF")
    make_identity(nc, identF)
    identB = const.tile([128, 128], BF16, tag="identB")
    make_identity(nc, identB)

    # ---- weights / cos / sin: DMA fp32 staging, cast to bf16 ----
    # wq and wk fused side by side: wqk[:, c, 0:192]=wq, [:, c, 192:384]=wk
    wqk = big.tile([128, KC, 2 * NG], BF16, tag="wqk")
    for i, ap in ((0, wq), (1, wk)):
        st = const.tile([128, KC, NG], F32, tag=f"st_w{i}", name=f"st_w{i}")
        nc.sync.dma_start(out=st, in_=ap.rearrange("(c p) n -> p c n", p=128))
        nc.vector.tensor_copy(wqk[:, :, ts(i, NG)], st)
    wv_st = const.tile([128, KC, NG], F32, tag="wv_st")
    nc.sync.dma_start(out=wv_st, in_=wv.rearrange("(c p) n -> p c n", p=128))
    wv_bf = big.tile([128, KC, NG], BF16, tag="wv_bf")
    nc.vector.tensor_copy(wv_bf, wv_st)

    wpa_st = const.tile([128, C], F32, tag="wpa_st")
    nc.sync.dma_start(out=wpa_st, in_=wp[0:128, :])
    wpa = big.tile([128, C], BF16, tag="wpa")
    nc.vector.tensor_copy(wpa, wpa_st)
    wpb_st = const.tile([64, C], F32, tag="wpb_st")
    nc.sync.dma_start(out=wpb_st, in_=wp[128:192, :])
    wpb = big.tile([64, C], BF16, tag="wpb")
    nc.vector.tensor_copy(wpb, wpb_st)

    cos_sb = big.tile([128, NT, 32], F32, tag="cos")
    nc.sync.dma_start(out=cos_sb, in_=cos.rearrange("(t p) d -> p t d", p=128))
    sin_sb = big.tile([128, NT, 32], F32, tag="sin")
    nc.sync.dma_start(out=sin_sb, in_=sin.rearrange("(t p) d -> p t d", p=128))

    # ---- persistent big tensors ----
    xT = big.tile([128, KC, T], BF16, tag="xT")
    qT01 = big.tile([128, T], BF16, tag="qT01")
    kT01 = big.tile([128, T], BF16, tag="kT01")
    qT22 = big.tile([128, T], BF16, tag="qT22")
    kT22 = big.tile([128, T], BF16, tag="kT22")
    yTa = big.tile([128, T], BF16, tag="yTa")   # rows 0:64 head0, 64:128 head1
    yTb = big.tile([64, T], BF16, tag="yTb")    # head2
    v_all = big.tile([128, NT, HL, 65], BF16, tag="v_all")
    q_stash = big.tile([128, NT, HL, 64], F32, tag="q_stash")
    nc.gpsimd.memset(v_all[:, :, :, 64:65], 1.0)

    work = ctx.enter_context(tc.tile_pool(name="work", bufs=4))

    def qk_side(src, t, d01, d22, psCp, pstag, d01_act, inv_pre=None):
        """rope + rmsnorm + head transposes for one of q/k.
        src: [128, HL, 64] fp32 SBUF AP for token tile t."""
        cos_t = cos_sb[:, t, :]
        sin_t = sin_sb[:, t, :]
        cos_b = bass.AP(tensor=cos_t.tensor, offset=cos_t.offset,
                        ap=[cos_t.ap[0], [0, HL], [0, 2], cos_t.ap[1]])
        sin_b = bass.AP(tensor=sin_t.tensor, offset=sin_t.offset,
                        ap=[sin_t.ap[0], [0, HL], [0, 2], sin_t.ap[1]])
        srcu = src.rearrange("p h (u d) -> p h u d", u=2)
        tcc = work.tile([128, HL, 2, 32], F32, tag="tcc",
                        name=f"tcc_{t}_{d01.name}")
        tss = work.tile([128, HL, 2, 32], F32, tag="tss",
                        name=f"tss_{t}_{d01.name}")
        qr = work.tile([128, HL, 64], F32, tag="qr", name=f"qr_{t}_{d01.name}")
        nc.vector.tensor_mul(tcc, srcu, cos_b)
        nc.vector.tensor_mul(tss, srcu, sin_b)
        nc.vector.tensor_add(qr[:, :, 0:32], tcc[:, :, 0, :], tss[:, :, 1, :])
        nc.vector.tensor_sub(qr[:, :, 32:64], tcc[:, :, 1, :],
                             tss[:, :, 0, :])

        # rmsnorm: inv = exp(-0.5*ln(sum(x^2)/64 + eps))
        if inv_pre is None:
            sq = work.tile([128, HL, 64], F32, tag="sq",
                           name=f"sq_{t}_{d01.name}")
            nc.vector.tensor_mul(sq, qr, qr)
            ms = work.tile([128, HL], F32, tag="ms",
                           name=f"ms_{t}_{d01.name}")
            nc.vector.reduce_sum(ms, sq, axis=mybir.AxisListType.X)
            mse = work.tile([128, HL], F32, tag="mse",
                            name=f"mse_{t}_{d01.name}")
            nc.vector.tensor_scalar_add(mse, ms, 64.0e-6)
            lms = work.tile([128, HL], F32, tag="lms",
                            name=f"lms_{t}_{d01.name}")
            nc.scalar.activation(lms, mse, AF.Ln, scale=1.0 / 64.0)
            inv = work.tile([128, HL], F32, tag="inv",
                            name=f"inv_{t}_{d01.name}")
            nc.scalar.activation(inv, lms, AF.Exp, scale=-0.5)
        else:
            inv = inv_pre
        qhat = work.tile([128, HL, 64], BF16, tag="qhat",
                         name=f"qhat_{t}_{d01.name}")
        for h in range(HL):
            nc.vector.tensor_scalar_mul(qhat[:, h, :], qr[:, h, :],
                                        inv[:, h:h + 1])

        # transpose heads: [tok, h*64] -> [d, tok]
        qhf = qhat.rearrange("p h d -> p (h d)")
        tp1 = psCp.tile([128, 2, 128], BF16, tag=pstag,
                        name=f"tp_{t}_{d01.name}")
        nc.tensor.transpose(tp1[:, 0, :], qhf[:, 0:128], identB)
        nc.tensor.transpose(tp1[0:64, 1, :], qhf[:, 128:192], identB)
        nc.tensor.transpose(tp1[64:128, 1, :], qhf[:, 128:192],
                            identB, tile_position=(0, 64))
        if d01_act:
            nc.scalar.copy(d01[:, ts(t, 128)], tp1[:, 0, :])
        else:
            nc.vector.tensor_copy(d01[:, ts(t, 128)], tp1[:, 0, :])
        nc.vector.tensor_copy(d22[:, ts(t, 128)], tp1[:, 1, :])

    # ===== pass 1: x load/transpose, QKV matmuls, k/v processing =====
    xpool = ctx.enter_context(tc.tile_pool(name="xin", bufs=3))
    with tc.tile_pool(name="psA", bufs=2, space="PSUM") as psA, \
         tc.tile_pool(name="psQK", bufs=2, space="PSUM") as psQK, \
         tc.tile_pool(name="psV", bufs=2, space="PSUM") as psV, \
         tc.tile_pool(name="psC", bufs=2, space="PSUM") as psC:
        for t in range(NT):
            x_t = xpool.tile([128, C], F32, tag="x_t")
            nc.sync.dma_start(out=x_t, in_=x[ts(t, 128), :])

            # transpose x tile -> xT (bf16 cast on the psum->sbuf copy)
            tpa = psA.tile([128, 4, 128], F32, tag="tpa", bufs=2)
            tpb = psA.tile([128, 2, 128], F32, tag="tpa", bufs=2,
                           name=f"tpb_{t}")
            for c in range(KC):
                dst = tpa[:, c, :] if c < 4 else tpb[:, c - 4, :]
                nc.tensor.transpose(dst, x_t[:, ts(c, 128)], identF)
            nc.scalar.copy(xT[:, 0:4, ts(t, 128)], tpa)
            nc.scalar.copy(xT[:, 4:6, ts(t, 128)], tpb)

            # QKV matmuls: q and k fused (N=384), v separate
            qk_ps = psQK.tile([128, 2, NG], F32, tag="qk_ps")
            v_ps = psV.tile([128, NG], F32, tag="v_ps")
            qk_f = qk_ps.rearrange("p a n -> p (a n)")
            for ci in range(KC):
                nc.tensor.matmul(qk_f, lhsT=xT[:, ci, ts(t, 128)],
                                 rhs=wqk[:, ci, :],
                                 start=(ci == 0), stop=(ci == KC - 1))
            for ci in range(KC):
                nc.tensor.matmul(v_ps, lhsT=xT[:, ci, ts(t, 128)],
                                 rhs=wv_bf[:, ci, :],
                                 start=(ci == 0), stop=(ci == KC - 1))

            # v: psum -> sbuf bf16 (ones column pre-set)
            v_ps3 = v_ps.rearrange("p (h d) -> p h d", h=HL)
            nc.scalar.copy(v_all[:, t, :, 0:64], v_ps3)

            # stash raw q for pass 2; stage k for immediate processing
            qk_ps4 = qk_ps.rearrange("p a (h d) -> p a h d", h=HL)
            nc.vector.tensor_copy(q_stash[:, t, :, :], qk_ps4[:, 0, :, :])
            qk_side(qk_ps4[:, 1, :, :], t, kT01, kT22, psC, "tpk",
                    d01_act=True)

    # ===== pass 2 + attention + projection, per tq chunk =====
    ppool = ctx.enter_context(tc.tile_pool(name="ppool", bufs=6))
    dn = ctx.enter_context(tc.tile_pool(name="dn", bufs=4))
    opool = ctx.enter_context(tc.tile_pool(name="opool", bufs=4))
    with tc.tile_pool(name="sps", bufs=2, space="PSUM") as sps, \
         tc.tile_pool(name="psY", bufs=1, space="PSUM") as psY, \
         tc.tile_pool(name="psP", bufs=1, space="PSUM") as psP:
        def q_chunk(tq):
            for tt in range(4):
                t = 4 * tq + tt
                qk_side(q_stash[:, t, :, :], t, qT01, qT22, psP, "aux",
                        d01_act=False)

        q_chunk(0)
        for tq in range(NTQ):
            tqs = ts(tq, TQB)
            # prepare the NEXT chunk's qT while this chunk's attention
            # (ACT-bound) runs; its psum slot frees early in this chunk.
            if tq + 1 < NTQ:
                q_chunk(tq + 1)

            yp = [psY.tile([65, TQB], F32, tag=f"yp{h}", bufs=1,
                           name=f"yp{h}_{tq}")
                  for h in range(HL)]
            for tkp in range(NT // 2):
                tk0, tk1 = 2 * tkp, 2 * tkp + 1
                # three 2-bank score tiles per tk pair, one fused exp each;
                # each tile's two matmuls use opposite PE row halves.
                for tag, mm in (
                    ("sA", ((kT01, qT01, slice(0, 64), tk0, None, 0),
                            (kT01, qT01, slice(64, 128), tk0, (64, 0), 1))),
                    ("sB", ((kT01, qT01, slice(0, 64), tk1, None, 0),
                            (kT01, qT01, slice(64, 128), tk1, (64, 0), 1))),
                    ("sC", ((kT22, qT22, slice(0, 64), tk0, None, 2),
                            (kT22, qT22, slice(64, 128), tk1, (64, 0), 2))),
                ):
                    s = sps.tile([128, 2, TQB], F32, tag="s",
                                 name=f"{tag}_{tq}_{tkp}")
                    for i, (kT, qT, half, tk, pos, _h) in enumerate(mm):
                        nc.tensor.matmul(s[:, i, :],
                                         lhsT=kT[half, ts(tk, 128)],
                                         rhs=qT[half, tqs],
                                         start=True, stop=True,
                                         tile_position=pos)
                    p = ppool.tile([128, 2, TQB], BF16, tag="p",
                                   name=f"p{tag}_{tq}_{tkp}")
                    nc.scalar.activation(p.rearrange("p a n -> p (a n)"),
                                         s.rearrange("p a n -> p (a n)"),
                                         AF.Exp, scale=0.125)
                    for i, (kT, qT, half, tk, pos, h) in enumerate(mm):
                        nc.tensor.matmul(yp[h], lhsT=v_all[:, tk, h, :],
                                         rhs=p[:, i, :],
                                         start=(tk == 0),
                                         stop=(tk == NT - 1))

            # normalize: row 64 of yp is the softmax denominator
            for h in range(HL):
                rec = dn.tile([1, TQB], F32, tag="rec", name=f"rec{h}_{tq}")
                nc.vector.reciprocal(rec, yp[h][64:65, :])
                rb = dn.tile([64, TQB], F32, tag="rb", name=f"rb{h}_{tq}")
                nc.gpsimd.partition_broadcast(rb, rec)
                if h == 0:
                    dst = yTa[0:64, tqs]
                elif h == 1:
                    dst = yTa[64:128, tqs]
                else:
                    dst = yTb[:, tqs]
                nc.vector.tensor_mul(dst, yp[h][0:64, :], rb)

            # projection for this tq chunk's 4 token tiles
            for tt in range(4):
                t = 4 * tq + tt
                o_sb = opool.tile([128, C], F32, tag="o_sb",
                                  name=f"o_sb_{t}")
                for nh in range(2):
                    nsl = ts(nh, 384)
                    pp = psP.tile([128, 384], F32, tag="aux",
                                  name=f"pp_{t}_{nh}")
                    nc.tensor.matmul(pp, lhsT=yTa[:, ts(t, 128)],
                                     rhs=wpa[:, nsl], start=True, stop=False)
                    nc.tensor.matmul(pp, lhsT=yTb[:, ts(t, 128)],
                                     rhs=wpb[:, nsl], start=False, stop=True)
                    nc.vector.tensor_copy(o_sb[:, nsl], pp)
                nc.sync.dma_start(out=y[ts(t, 128), :], in_=o_sb)


def build_nc(reps=1):
    nc = bacc.Bacc("TRN2", target_bir_lowering=False, debug=False,
                   num_devices=8)
    x = nc.dram_tensor("x", [T, C], F32, kind="ExternalInput").ap()
    cos = nc.dram_tensor("cos", [T, 32], F32, kind="ExternalInput").ap()
    sin = nc.dram_tensor("sin", [T, 32], F32, kind="ExternalInput").ap()
    wq = nc.dram_tensor("wq", [C, NG], F32, kind="ExternalInput").ap()
    wk = nc.dram_tensor("wk", [C, NG], F32, kind="ExternalInput").ap()
    wv = nc.dram_tensor("wv", [C, NG], F32, kind="ExternalInput").ap()
    wp = nc.dram_tensor("wp", [NG, C], F32, kind="ExternalInput").ap()
    y = nc.dram_tensor("y", [T, C], F32, kind="ExternalOutput").ap()
    with tile.TileContext(nc) as tc:
        for _ in range(reps):
            with ExitStack() as ctx:
                build_kernel(tc, ctx, x, cos, sin, wq, wk, wv, wp, y)
    nc.compile()
    return nc


def make_in_maps(x, cos, sin, wq, wk, wv, wproj):
    cos2 = np.ascontiguousarray(np.asarray(cos, np.float32).reshape(T, 32))
    sin2 = np.ascontiguousarray(np.asarray(sin, np.float32).reshape(T, 32))
    in_maps = []
    for cid in range(8):
        b, g = divmod(cid, 4)
        sl = slice(g * NG, (g + 1) * NG)
        in_maps.append({
            "x": np.ascontiguousarray(np.asarray(x, np.float32)[b]),
            "cos": cos2,
            "sin": sin2,
            "wq": np.ascontiguousarray(np.asarray(wq, np.float32)[:, sl]),
            "wk": np.ascontiguousarray(np.asarray(wk, np.float32)[:, sl]),
            "wv": np.ascontiguousarray(np.asarray(wv, np.float32)[:, sl]),
            "wp": np.ascontiguousarray(np.asarray(wproj, np.float32)[sl, :]),
        })
    return in_maps


_NC = None


def kernel(x, cos, sin, wq, wk, wv, wproj):
    global _NC
    if _NC is None:
        _NC = build_nc()
    in_maps = make_in_maps(x, cos, sin, wq, wk, wv, wproj)
    res = run_bass_kernel_spmd(_NC, in_maps, list(range(8)))
    outs = [r["y"].astype(np.float64) for r in res.results]
    y0 = outs[0] + outs[1] + outs[2] + outs[3]
    y1 = outs[4] + outs[5] + outs[6] + outs[7]
    return np.stack([y0, y1], axis=0).astype(np.float32)


if __name__ == "__main__":
    rng = np.random.default_rng(0)
    ins = {
        "x": rng.standard_normal((2, T, C), dtype=np.float32),
        "cos": rng.random((T, 1, 32), dtype=np.float32),
        "sin": rng.random((T, 1, 32), dtype=np.float32),
        "wq": rng.standard_normal((C, C), dtype=np.float32) / np.sqrt(C),
        "wk": rng.standard_normal((C, C), dtype=np.float32) / np.sqrt(C),
        "wv": rng.standard_normal((C, C), dtype=np.float32) / np.sqrt(C),
        "wproj": rng.standard_normal((C, C), dtype=np.float32) / np.sqrt(C),
    }
    out = kernel(**ins)
    print(out.shape, out.dtype, np.abs(out).max())



# revision 17
# speedup vs baseline: 2.6906x; 2.6906x over previous
# Multi-head attention (B=2, T=2048, C=768, H=12, D=64) on 8 NeuronCores.
#
# Sharding: core i handles batch b = i // 4 and head group g = i % 4
# (3 heads each).  Host pre-transposes/casts inputs; each core computes
# q/k DIRECTLY in transposed [d, token] layout (lhsT = weight chunk,
# rhs = xT chunk), so no PE transposes are needed anywhere:
#   qT/kT[hd, tok] = sum_ci wq[ci*128:+128, hd]^T @ xT[ci, tok]
#   rope in transposed layout with host-duplicated cosT/sinT tables
#   rmsnorm via block-ones PE matmul (partition reduction) + Ln/Exp
#   scores s^T [tk,tq] = kT.T @ qT ; p = exp(s/8) (fused 2-head tiles)
#   AV: yplus += [v | 1].T @ p  (v computed in [tok, hd] layout)
#   softmax denom = ones-row of yplus; reciprocal + partition-broadcast
#   proj: out = yT.T @ wp slices -> bf16 partials, summed on host.

import numpy as np
from contextlib import ExitStack
import ml_dtypes

import concourse.hw_specs as _hw_specs
from concourse import mybir

AF = mybir.ActivationFunctionType
ALU = mybir.AluOpType

# Keep Exp/Ln in exactly one ACT table set so bacc's greedy set selection
# never bounces between table sets (each bounce is a ~1.3us table DMA).
if not getattr(_hw_specs, "_mha_act_patch", False):
    _orig_gat = _hw_specs.get_activation_tables

    def _gat_one_exp_ln_set(arch):
        tabs = _orig_gat(arch)
        for name, s in tabs.items():
            if name != "natural_log_exp_and_others":
                s.discard(AF.Exp)
                s.discard(AF.Ln)
        return tabs

    _hw_specs.get_activation_tables = _gat_one_exp_ln_set
    _hw_specs._mha_act_patch = True

import concourse.bass as bass          # noqa: E402
import concourse.tile as tile          # noqa: E402
from concourse import bacc             # noqa: E402
bacc.get_activation_tables = _hw_specs.get_activation_tables
from concourse.bass import ts          # noqa: E402
from concourse.bass_utils import run_bass_kernel_spmd  # noqa: E402

F32 = mybir.dt.float32
BF16 = mybir.dt.bfloat16
BF16NP = ml_dtypes.bfloat16

T = 2048
C = 768
HL = 3          # heads per core
D = 64
NG = HL * D     # 192, per-core qkv width
NT = T // 128   # 16 token tiles
KC = C // 128   # 6 contraction chunks
TQB = 512       # tq block
NTQ = T // TQB  # 4


def build_kernel(tc, ctx, xT, cosd, sind, wq, wk, wv, wpa, wpb, y):
    nc = tc.nc

    big = ctx.enter_context(tc.tile_pool(name="big", bufs=1))

    # ---- persistent inputs: one DMA each, already bf16/transposed ----
    xTs = big.tile([128, KC, T], BF16, tag="xTs")
    nc.sync.dma_start(out=xTs, in_=xT)
    wqs = big.tile([128, KC, NG], BF16, tag="wqs")
    nc.sync.dma_start(out=wqs, in_=wq)
    wks = big.tile([128, KC, NG], BF16, tag="wks")
    nc.sync.dma_start(out=wks, in_=wk)
    wvs = big.tile([128, KC, NG], BF16, tag="wvs")
    nc.sync.dma_start(out=wvs, in_=wv)
    wpa_s = big.tile([128, C], BF16, tag="wpa_s")
    nc.sync.dma_start(out=wpa_s, in_=wpa)
    wpb_s = big.tile([64, C], BF16, tag="wpb_s")
    nc.sync.dma_start(out=wpb_s, in_=wpb)
    # cos/sin arrive as [32, T]; replicate to 4 row-blocks on device.
    # sin is stored SIGNED: rows j<32 = -sin (for the y2 = x2*c - x1*s
    # half after the 32-row swap), rows 32:64 = +sin; pattern repeats.
    cosd_s = big.tile([128, T], F32, tag="cosd_s")
    nc.sync.dma_start(out=cosd_s[0:32, :], in_=cosd)
    sind_s = big.tile([128, T], F32, tag="sind_s")
    nc.sync.dma_start(out=sind_s[32:64, :], in_=sind)
    nc.vector.tensor_scalar_mul(sind_s[0:32, :], sind_s[32:64, :], -1.0)
    for r in range(1, 4):
        nc.vector.tensor_copy(cosd_s[ts(r, 32), :], cosd_s[0:32, :])
    nc.vector.tensor_copy(sind_s[64:128, :], sind_s[0:64, :])

    # block-ones for the rmsnorm partition reduction; M=64-wide so the
    # reduction matmul also BROADCASTS the per-head sum to 64 rows
    onesH0 = big.tile([128, 64], BF16, tag="onesH0")
    nc.gpsimd.memset(onesH0, 0.0)
    nc.gpsimd.memset(onesH0[0:64, :], 1.0)
    onesH1 = big.tile([128, 64], BF16, tag="onesH1")
    nc.gpsimd.memset(onesH1, 0.0)
    nc.gpsimd.memset(onesH1[64:128, :], 1.0)
    onesBB = big.tile([64, 64], BF16, tag="onesBB")
    nc.gpsimd.memset(onesBB, 1.0)

    # ---- persistent big tensors ----
    qT01 = big.tile([128, T], BF16, tag="qT01")
    kT01 = big.tile([128, T], BF16, tag="kT01")
    qT22 = big.tile([128, T], BF16, tag="qT22")
    kT22 = big.tile([128, T], BF16, tag="kT22")
    yTa = big.tile([128, T], BF16, tag="yTa")   # rows 0:64 head0, 64:128 h1
    yTb = big.tile([64, T], BF16, tag="yTb")    # head2
    v_all = big.tile([128, NT, HL, 65], BF16, tag="v_all")
    nc.gpsimd.memset(v_all[:, :, :, 64:65], 1.0)

    work = ctx.enter_context(tc.tile_pool(name="work", bufs=1))
    dnq = ctx.enter_context(tc.tile_pool(name="dnq", bufs=1))
    dn = ctx.enter_context(tc.tile_pool(name="dn", bufs=2))

    # ===== pass 1: qT/kT blocks (transposed-direct) + v tiles =====
    QB = 1024   # qk processing block width (2 psum banks)
    probe = {}
    if getattr(tc, "_mha_probe", False):
        for pn, shape, dt in (("pA", [128, QB], F32), ("yrA", [128, QB], F32),
                              ("sqA", [128, QB], BF16),
                              ("ms", [128, QB], F32),
                              ("ib", [128, QB], F32)):
            probe[pn] = big.tile(shape, dt, tag=f"probe_{pn}", name=f"probe_{pn}")
    with tc.tile_pool(name="psA", bufs=1, space="PSUM") as psA, \
         tc.tile_pool(name="psB", bufs=1, space="PSUM") as psB, \
         tc.tile_pool(name="psM", bufs=1, space="PSUM") as psM, \
         tc.tile_pool(name="psMB", bufs=1, space="PSUM") as psMB:

        def qk_block(w_s, blk, d01, d22):
            blks = ts(blk, QB)
            nm = f"{d01.name}_{blk}"
            pA = psA.tile([128, QB], F32, tag="pA", name=f"pA_{nm}")
            pB = psB.tile([64, QB], F32, tag="pB", name=f"pB_{nm}")
            for half in range(QB // 512):
                hs = ts(half, 512)
                bs = slice(blk * QB + half * 512, blk * QB + half * 512 + 512)
                for ci in range(KC):
                    nc.tensor.matmul(pA[:, hs], lhsT=w_s[:, ci, 0:128],
                                     rhs=xTs[:, ci, bs],
                                     start=(ci == 0), stop=(ci == KC - 1))
                for ci in range(KC):
                    nc.tensor.matmul(pB[:, hs], lhsT=w_s[:, ci, 128:NG],
                                     rhs=xTs[:, ci, bs],
                                     start=(ci == 0), stop=(ci == KC - 1))
            # rope: rows j<32 of each 64-row head block are x1, j>=32 x2.
            # u = x * signed_sin; swap u's 32-row halves (dst-shifted
            # copies are legal, misaligned-src adds are not), then one
            # aligned add: yr = x*cos + swap(u).
            tcA = work.tile([128, QB], F32, tag="tcA", name=f"tcA_{nm}")
            uA = work.tile([128, QB], F32, tag="uA", name=f"uA_{nm}")
            nc.vector.tensor_mul(tcA, pA, cosd_s[:, blks])
            nc.vector.tensor_mul(uA, pA, sind_s[:, blks])
            tcB = work.tile([64, QB], F32, tag="tcB", name=f"tcB_{nm}")
            uB = work.tile([64, QB], F32, tag="uB", name=f"uB_{nm}")
            nc.vector.tensor_mul(tcB, pB, cosd_s[0:64, blks])
            nc.vector.tensor_mul(uB, pB, sind_s[0:64, blks])
            uSwA = work.tile([128, QB], F32, tag="uSwA", name=f"uSwA_{nm}")
            uSwB = work.tile([64, QB], F32, tag="uSwB", name=f"uSwB_{nm}")
            nc.vector.tensor_copy(uSwA[0:32], uA[32:64])
            nc.vector.tensor_copy(uSwA[32:64], uA[0:32])
            nc.vector.tensor_copy(uSwA[64:96], uA[96:128])
            nc.vector.tensor_copy(uSwA[96:128], uA[64:96])
            nc.vector.tensor_copy(uSwB[0:32], uB[32:64])
            nc.vector.tensor_copy(uSwB[32:64], uB[0:32])
            yrA = work.tile([128, QB], F32, tag="yrA", name=f"yrA_{nm}")
            yrB = work.tile([64, QB], F32, tag="yrB", name=f"yrB_{nm}")
            nc.vector.tensor_add(yrA, tcA, uSwA)
            nc.vector.tensor_add(yrB, tcB, uSwB)
            # rmsnorm: per-head partition reduction of squares via PE
            sqA = work.tile([128, QB], BF16, tag="sqA", name=f"sqA_{nm}")
            sqB = work.tile([64, QB], BF16, tag="sqB", name=f"sqB_{nm}")
            nc.vector.tensor_mul(sqA, yrA, yrA)
            nc.vector.tensor_mul(sqB, yrB, yrB)
            # per-head sum-of-squares, broadcast to all 64 head rows by
            # the 64-wide block-ones lhsT (no partition_broadcast needed)
            ms = psM.tile([128, QB], F32, tag="ms", name=f"ms_{nm}")
            msB = psMB.tile([64, QB], F32, tag="msB", name=f"msB_{nm}")
            for half in range(QB // 512):
                hs = ts(half, 512)
                nc.tensor.matmul(ms[0:64, hs], lhsT=onesH0,
                                 rhs=sqA[:, hs], start=True, stop=True)
                nc.tensor.matmul(ms[64:128, hs], lhsT=onesH1,
                                 rhs=sqA[:, hs], start=True, stop=True)
                nc.tensor.matmul(msB[:, hs], lhsT=onesBB, rhs=sqB[:, hs],
                                 start=True, stop=True)
            mse = dnq.tile([128, QB], F32, tag="mse", name=f"mse_{nm}")
            nc.vector.tensor_scalar_add(mse, ms, 64.0e-6)
            lms = dnq.tile([128, QB], F32, tag="lms", name=f"lms_{nm}")
            nc.scalar.activation(lms, mse, AF.Ln, scale=1.0 / 64.0)
            ib = dnq.tile([128, QB], F32, tag="ib", name=f"ib_{nm}")
            nc.scalar.activation(ib, lms, AF.Exp, scale=-0.5)
            mseB = dnq.tile([64, QB], F32, tag="mseB", name=f"mseB_{nm}")
            nc.vector.tensor_scalar_add(mseB, msB, 64.0e-6)
            lmsB = dnq.tile([64, QB], F32, tag="lmsB", name=f"lmsB_{nm}")
            nc.scalar.activation(lmsB, mseB, AF.Ln, scale=1.0 / 64.0)
            ibB = dnq.tile([64, QB], F32, tag="ibB", name=f"ibB_{nm}")
            nc.scalar.activation(ibB, lmsB, AF.Exp, scale=-0.5)
            nc.vector.tensor_mul(d01[:, blks], yrA, ib)
            nc.vector.tensor_mul(d22[0:64, blks], yrB, ibB)
            nc.vector.tensor_mul(d22[64:128, blks], yrB, ibB)
            if probe and blk == 0 and d01 is qT01:
                nc.vector.tensor_copy(probe["pA"], pA)
                nc.vector.tensor_copy(probe["yrA"], yrA)
                nc.vector.tensor_copy(probe["sqA"], sqA)
                nc.vector.tensor_copy(probe["ms"], ms)
                nc.vector.tensor_copy(probe["ib"], ib)

        for blk in range(T // QB):
            qk_block(wqs, blk, qT01, qT22)
            qk_block(wks, blk, kT01, kT22)

    with tc.tile_pool(name="psV", bufs=2, space="PSUM") as psV:
        for t in range(NT):
            v_ps = psV.tile([128, NG], F32, tag="v_ps", name=f"v_ps_{t}")
            for ci in range(KC):
                nc.tensor.matmul(v_ps, lhsT=xTs[:, ci, ts(t, 128)],
                                 rhs=wvs[:, ci, :],
                                 start=(ci == 0), stop=(ci == KC - 1))
            v_ps3 = v_ps.rearrange("p (h d) -> p h d", h=HL)
            nc.scalar.copy(v_all[:, t, :, 0:64], v_ps3)

    # ===== pass 2: attention + projection, per tq chunk =====
    ppool = ctx.enter_context(tc.tile_pool(name="ppool", bufs=3))
    opool = ctx.enter_context(tc.tile_pool(name="opool", bufs=4))
    with tc.tile_pool(name="sps", bufs=1, space="PSUM") as sps, \
         tc.tile_pool(name="psY", bufs=1, space="PSUM") as psY:
        for tq in range(NTQ):
            tqs = ts(tq, TQB)
            yp = [psY.tile([65, TQB], F32, tag=f"yp{h}", bufs=1,
                           name=f"yp{h}_{tq}")
                  for h in range(HL)]
            for g in range(NT // 4):
                tkg = [4 * g + j for j in range(4)]
                # three 4-plane score tiles per 4-tk group, one fused exp
                # each; paired planes use opposite PE row halves.
                for tag, mm in (
                    ("sa", ((kT01, qT01, slice(0, 64), tkg[0], None, 0),
                            (kT01, qT01, slice(64, 128), tkg[0], (64, 0), 1),
                            (kT01, qT01, slice(0, 64), tkg[1], None, 0),
                            (kT01, qT01, slice(64, 128), tkg[1], (64, 0), 1))),
                    ("sb", ((kT01, qT01, slice(0, 64), tkg[2], None, 0),
                            (kT01, qT01, slice(64, 128), tkg[2], (64, 0), 1),
                            (kT01, qT01, slice(0, 64), tkg[3], None, 0),
                            (kT01, qT01, slice(64, 128), tkg[3], (64, 0), 1))),
                    ("sc", ((kT22, qT22, slice(0, 64), tkg[0], None, 2),
                            (kT22, qT22, slice(64, 128), tkg[1], (64, 0), 2),
                            (kT22, qT22, slice(0, 64), tkg[2], None, 2),
                            (kT22, qT22, slice(64, 128), tkg[3], (64, 0), 2))),
                ):
                    s = sps.tile([128, 4, TQB], F32, tag="s4",
                                 name=f"{tag}_{tq}_{g}")
                    for i, (kT, qT, half, tk, pos, _h) in enumerate(mm):
                        nc.tensor.matmul(s[:, i, :],
                                         lhsT=kT[half, ts(tk, 128)],
                                         rhs=qT[half, tqs],
                                         start=True, stop=True,
                                         tile_position=pos)
                    p = ppool.tile([128, 4, TQB], BF16, tag="p",
                                   name=f"p{tag}_{tq}_{g}")
                    nc.scalar.activation(p.rearrange("p a n -> p (a n)"),
                                         s.rearrange("p a n -> p (a n)"),
                                         AF.Exp, scale=0.125)
                    for i, (kT, qT, half, tk, pos, h) in enumerate(mm):
                        nc.tensor.matmul(yp[h], lhsT=v_all[:, tk, h, :],
                                         rhs=p[:, i, :],
                                         start=(tk == 0),
                                         stop=(tk == NT - 1))

            # normalize: row 64 of yp is the softmax denominator
            for h in range(HL):
                rec = dn.tile([1, TQB], F32, tag="rec", name=f"rec{h}_{tq}")
                nc.vector.reciprocal(rec, yp[h][64:65, :])
                rb = dn.tile([64, TQB], F32, tag="rb", name=f"rb{h}_{tq}")
                nc.gpsimd.partition_broadcast(rb, rec)
                if h == 0:
                    dst = yTa[0:64, tqs]
                elif h == 1:
                    dst = yTa[64:128, tqs]
                else:
                    dst = yTb[:, tqs]
                nc.vector.tensor_mul(dst, yp[h][0:64, :], rb)

            # projection for this tq chunk's 4 token tiles (bf16 partials)
            for tt in range(4):
                t = 4 * tq + tt
                o_sb = opool.tile([128, C], BF16, tag="o_sb",
                                  name=f"o_sb_{t}")
                # plane stride padded to 512 so each matmul dst is
                # PSUM-bank aligned (384 f32 would straddle banks)
                pp = sps.tile([128, 2, 512], F32, tag="s4",
                              name=f"pp_{t}")
                for nh in range(2):
                    nc.tensor.matmul(pp[:, nh, 0:384],
                                     lhsT=yTa[:, ts(t, 128)],
                                     rhs=wpa_s[:, ts(nh, 384)], start=True,
                                     stop=False)
                    nc.tensor.matmul(pp[:, nh, 0:384],
                                     lhsT=yTb[:, ts(t, 128)],
                                     rhs=wpb_s[:, ts(nh, 384)], start=False,
                                     stop=True)
                nc.vector.tensor_copy(
                    o_sb.rearrange("p (a n) -> p a n", a=2),
                    pp[:, :, 0:384])
                nc.sync.dma_start(out=y[ts(t, 128), :], in_=o_sb)

    return (qT01, qT22, kT01, v_all, yTa, yTb, cosd_s, sind_s, probe)


def build_nc(reps=1):
    nc = bacc.Bacc("TRN2", target_bir_lowering=False, debug=False,
                   num_devices=8)
    xT = nc.dram_tensor("xT", [128, KC, T], BF16, kind="ExternalInput").ap()
    cosd = nc.dram_tensor("cosd", [32, T], F32, kind="ExternalInput").ap()
    sind = nc.dram_tensor("sind", [32, T], F32, kind="ExternalInput").ap()
    wq = nc.dram_tensor("wq", [128, KC, NG], BF16, kind="ExternalInput").ap()
    wk = nc.dram_tensor("wk", [128, KC, NG], BF16, kind="ExternalInput").ap()
    wv = nc.dram_tensor("wv", [128, KC, NG], BF16, kind="ExternalInput").ap()
    wpa = nc.dram_tensor("wpa", [128, C], BF16, kind="ExternalInput").ap()
    wpb = nc.dram_tensor("wpb", [64, C], BF16, kind="ExternalInput").ap()
    y = nc.dram_tensor("y", [T, C], BF16, kind="ExternalOutput").ap()
    with tile.TileContext(nc) as tc:
        for _ in range(reps):
            with ExitStack() as ctx:
                build_kernel(tc, ctx, xT, cosd, sind, wq, wk, wv,
                             wpa, wpb, y)
    nc.compile()
    return nc


def make_in_maps(x, cos, sin, wq, wk, wv, wproj):
    x = np.asarray(x, np.float32)
    cosd = np.ascontiguousarray(
        np.asarray(cos, np.float32).reshape(T, 32).T)      # [32, T]
    sind = np.ascontiguousarray(
        np.asarray(sin, np.float32).reshape(T, 32).T)
    wq = np.asarray(wq, np.float32)
    wk = np.asarray(wk, np.float32)
    wv = np.asarray(wv, np.float32)
    wp = np.asarray(wproj, np.float32)

    def to_pcn(w):  # [768, n] f32 -> [128, 6, n] bf16
        n = w.shape[1]
        return np.ascontiguousarray(
            w.reshape(KC, 128, n).transpose(1, 0, 2)).astype(BF16NP)

    in_maps = []
    for cid in range(8):
        b, g = divmod(cid, 4)
        sl = slice(g * NG, (g + 1) * NG)
        xTb = np.ascontiguousarray(
            x[b].T.reshape(KC, 128, T).transpose(1, 0, 2)).astype(BF16NP)
        in_maps.append({
            "xT": xTb,
            "cosd": cosd,
            "sind": sind,
            "wq": to_pcn(wq[:, sl]),
            "wk": to_pcn(wk[:, sl]),
            "wv": to_pcn(wv[:, sl]),
            "wpa": np.ascontiguousarray(
                wp[g * NG:g * NG + 128, :]).astype(BF16NP),
            "wpb": np.ascontiguousarray(
                wp[g * NG + 128:(g + 1) * NG, :]).astype(BF16NP),
        })
    return in_maps


_NC = None


def kernel(x, cos, sin, wq, wk, wv, wproj):
    global _NC
    if _NC is None:
        _NC = build_nc()
    in_maps = make_in_maps(x, cos, sin, wq, wk, wv, wproj)
    res = run_bass_kernel_spmd(_NC, in_maps, list(range(8)))
    outs = [r["y"].astype(np.float32) for r in res.results]
    y0 = outs[0] + outs[1] + outs[2] + outs[3]
    y1 = outs[4] + outs[5] + outs[6] + outs[7]
    return np.stack([y0, y1], axis=0).astype(np.float32)


if __name__ == "__main__":
    rng = np.random.default_rng(0)
    ins = {
        "x": rng.standard_normal((2, T, C), dtype=np.float32),
        "cos": rng.random((T, 1, 32), dtype=np.float32),
        "sin": rng.random((T, 1, 32), dtype=np.float32),
        "wq": rng.standard_normal((C, C), dtype=np.float32) / np.sqrt(C),
        "wk": rng.standard_normal((C, C), dtype=np.float32) / np.sqrt(C),
        "wv": rng.standard_normal((C, C), dtype=np.float32) / np.sqrt(C),
        "wproj": rng.standard_normal((C, C), dtype=np.float32) / np.sqrt(C),
    }
    out = kernel(**ins)
    print(out.shape, out.dtype, np.abs(out).max())


# revision 19
# speedup vs baseline: 2.6921x; 1.0006x over previous
# Multi-head attention (B=2, T=2048, C=768, H=12, D=64) on 8 NeuronCores.
#
# Sharding: core i handles batch b = i // 4 and head group g = i % 4
# (3 heads each).  Host pre-transposes/casts inputs; each core computes
# q/k DIRECTLY in transposed [d, token] layout (lhsT = weight chunk,
# rhs = xT chunk), so no PE transposes are needed anywhere:
#   qT/kT[hd, tok] = sum_ci wq[ci*128:+128, hd]^T @ xT[ci, tok]
#   rope in transposed layout with host-duplicated cosT/sinT tables
#   rmsnorm via block-ones PE matmul (partition reduction) + Ln/Exp
#   scores s^T [tk,tq] = kT.T @ qT ; p = exp(s/8) (fused 2-head tiles)
#   AV: yplus += [v | 1].T @ p  (v computed in [tok, hd] layout)
#   softmax denom = ones-row of yplus; reciprocal + partition-broadcast
#   proj: out = yT.T @ wp slices -> bf16 partials, summed on host.

import numpy as np
from contextlib import ExitStack
import ml_dtypes

import concourse.hw_specs as _hw_specs
from concourse import mybir

AF = mybir.ActivationFunctionType
ALU = mybir.AluOpType

# Keep Exp/Ln in exactly one ACT table set so bacc's greedy set selection
# never bounces between table sets (each bounce is a ~1.3us table DMA).
if not getattr(_hw_specs, "_mha_act_patch", False):
    _orig_gat = _hw_specs.get_activation_tables

    def _gat_one_exp_ln_set(arch):
        tabs = _orig_gat(arch)
        for name, s in tabs.items():
            if name != "natural_log_exp_and_others":
                s.discard(AF.Exp)
                s.discard(AF.Ln)
        return tabs

    _hw_specs.get_activation_tables = _gat_one_exp_ln_set
    _hw_specs._mha_act_patch = True

import concourse.bass as bass          # noqa: E402
import concourse.tile as tile          # noqa: E402
from concourse import bacc             # noqa: E402
bacc.get_activation_tables = _hw_specs.get_activation_tables
from concourse.bass import ts          # noqa: E402
from concourse.bass_utils import run_bass_kernel_spmd  # noqa: E402

F32 = mybir.dt.float32
BF16 = mybir.dt.bfloat16
BF16NP = ml_dtypes.bfloat16

T = 2048
C = 768
HL = 3          # heads per core
D = 64
NG = HL * D     # 192, per-core qkv width
NT = T // 128   # 16 token tiles
KC = C // 128   # 6 contraction chunks
TQB = 512       # tq block
NTQ = T // TQB  # 4


def build_kernel(tc, ctx, xT, cosd, sind, wq, wk, wv, wpa, wpb, y):
    nc = tc.nc

    big = ctx.enter_context(tc.tile_pool(name="big", bufs=1))

    # ---- persistent inputs: one DMA each, already bf16/transposed ----
    xTs = big.tile([128, KC, T], BF16, tag="xTs")
    nc.sync.dma_start(out=xTs, in_=xT)
    ws = big.tile([128, KC, 3 * NG], BF16, tag="ws")
    nc.sync.dma_start(out=ws, in_=wq)   # wq dram tensor holds [wq|wk|wv]
    wqs = ws[:, :, 0:NG]
    wks = ws[:, :, NG:2 * NG]
    wvs = ws[:, :, 2 * NG:3 * NG]
    wpa_s = big.tile([128, C], BF16, tag="wpa_s")
    nc.sync.dma_start(out=wpa_s, in_=wpa)
    wpb_s = big.tile([64, C], BF16, tag="wpb_s")
    nc.sync.dma_start(out=wpb_s, in_=wpb)
    # cos/sin arrive as [32, T]; replicate to 4 row-blocks on device.
    # sin is stored SIGNED: rows j<32 = -sin (for the y2 = x2*c - x1*s
    # half after the 32-row swap), rows 32:64 = +sin; pattern repeats.
    cosd_s = big.tile([128, T], F32, tag="cosd_s")
    nc.sync.dma_start(out=cosd_s[0:32, :], in_=cosd)
    sind_s = big.tile([128, T], F32, tag="sind_s")
    nc.sync.dma_start(out=sind_s[32:64, :], in_=sind)
    nc.vector.tensor_scalar_mul(sind_s[0:32, :], sind_s[32:64, :], -1.0)
    for r in range(1, 4):
        nc.vector.tensor_copy(cosd_s[ts(r, 32), :], cosd_s[0:32, :])
    nc.vector.tensor_copy(sind_s[64:128, :], sind_s[0:64, :])

    # block-ones for the rmsnorm partition reduction; M=64-wide so the
    # reduction matmul also BROADCASTS the per-head sum to 64 rows
    onesH0 = big.tile([128, 64], BF16, tag="onesH0")
    nc.gpsimd.memset(onesH0, 0.0)
    nc.gpsimd.memset(onesH0[0:64, :], 1.0)
    onesH1 = big.tile([128, 64], BF16, tag="onesH1")
    nc.gpsimd.memset(onesH1, 0.0)
    nc.gpsimd.memset(onesH1[64:128, :], 1.0)
    onesBB = big.tile([64, 64], BF16, tag="onesBB")
    nc.gpsimd.memset(onesBB, 1.0)

    # ---- persistent big tensors ----
    qT01 = big.tile([128, T], BF16, tag="qT01")
    kT01 = big.tile([128, T], BF16, tag="kT01")
    qT22 = big.tile([128, T], BF16, tag="qT22")
    kT22 = big.tile([128, T], BF16, tag="kT22")
    yTa = big.tile([128, T], BF16, tag="yTa")   # rows 0:64 head0, 64:128 h1
    yTb = big.tile([64, T], BF16, tag="yTb")    # head2
    v_all = big.tile([128, NT, HL, 65], BF16, tag="v_all")
    nc.gpsimd.memset(v_all[:, :, :, 64:65], 1.0)

    work = ctx.enter_context(tc.tile_pool(name="work", bufs=1))
    dnq = ctx.enter_context(tc.tile_pool(name="dnq", bufs=1))
    dn = ctx.enter_context(tc.tile_pool(name="dn", bufs=2))

    # ===== pass 1: qT/kT blocks (transposed-direct) + v tiles =====
    QB = 1024   # qk processing block width (2 psum banks)
    probe = {}
    if getattr(tc, "_mha_probe", False):
        for pn, shape, dt in (("pA", [128, QB], F32), ("yrA", [128, QB], F32),
                              ("sqA", [128, QB], BF16),
                              ("ms", [128, QB], F32),
                              ("ib", [128, QB], F32)):
            probe[pn] = big.tile(shape, dt, tag=f"probe_{pn}", name=f"probe_{pn}")
    with tc.tile_pool(name="psA", bufs=1, space="PSUM") as psA, \
         tc.tile_pool(name="psB", bufs=1, space="PSUM") as psB, \
         tc.tile_pool(name="psM", bufs=1, space="PSUM") as psM, \
         tc.tile_pool(name="psMB", bufs=1, space="PSUM") as psMB:

        def qk_block(w_s, blk, d01, d22):
            blks = ts(blk, QB)
            nm = f"{d01.name}_{blk}"
            pA = psA.tile([128, QB], F32, tag="pA", name=f"pA_{nm}")
            pB = psB.tile([64, QB], F32, tag="pB", name=f"pB_{nm}")
            for half in range(QB // 512):
                hs = ts(half, 512)
                bs = slice(blk * QB + half * 512, blk * QB + half * 512 + 512)
                for ci in range(KC):
                    nc.tensor.matmul(pA[:, hs], lhsT=w_s[:, ci, 0:128],
                                     rhs=xTs[:, ci, bs],
                                     start=(ci == 0), stop=(ci == KC - 1))
                for ci in range(KC):
                    nc.tensor.matmul(pB[:, hs], lhsT=w_s[:, ci, 128:NG],
                                     rhs=xTs[:, ci, bs],
                                     start=(ci == 0), stop=(ci == KC - 1))
            # rope: rows j<32 of each 64-row head block are x1, j>=32 x2.
            # u = x * signed_sin; swap u's 32-row halves (dst-shifted
            # copies are legal, misaligned-src adds are not), then one
            # aligned add: yr = x*cos + swap(u).
            tcA = work.tile([128, QB], F32, tag="tcA", name=f"tcA_{nm}")
            uA = work.tile([128, QB], F32, tag="uA", name=f"uA_{nm}")
            nc.vector.tensor_mul(tcA, pA, cosd_s[:, blks])
            nc.vector.tensor_mul(uA, pA, sind_s[:, blks])
            tcB = work.tile([64, QB], F32, tag="tcB", name=f"tcB_{nm}")
            uB = work.tile([64, QB], F32, tag="uB", name=f"uB_{nm}")
            nc.vector.tensor_mul(tcB, pB, cosd_s[0:64, blks])
            nc.vector.tensor_mul(uB, pB, sind_s[0:64, blks])
            uSwA = work.tile([128, QB], F32, tag="uSwA", name=f"uSwA_{nm}")
            uSwB = work.tile([64, QB], F32, tag="uSwB", name=f"uSwB_{nm}")
            nc.vector.tensor_copy(uSwA[0:32], uA[32:64])
            nc.vector.tensor_copy(uSwA[32:64], uA[0:32])
            nc.vector.tensor_copy(uSwA[64:96], uA[96:128])
            nc.vector.tensor_copy(uSwA[96:128], uA[64:96])
            nc.vector.tensor_copy(uSwB[0:32], uB[32:64])
            nc.vector.tensor_copy(uSwB[32:64], uB[0:32])
            yrA = work.tile([128, QB], F32, tag="yrA", name=f"yrA_{nm}")
            yrB = work.tile([64, QB], F32, tag="yrB", name=f"yrB_{nm}")
            nc.vector.tensor_add(yrA, tcA, uSwA)
            nc.vector.tensor_add(yrB, tcB, uSwB)
            # rmsnorm: per-head partition reduction of squares via PE
            sqA = work.tile([128, QB], BF16, tag="sqA", name=f"sqA_{nm}")
            sqB = work.tile([64, QB], BF16, tag="sqB", name=f"sqB_{nm}")
            nc.vector.tensor_mul(sqA, yrA, yrA)
            nc.vector.tensor_mul(sqB, yrB, yrB)
            # per-head sum-of-squares, broadcast to all 64 head rows by
            # the 64-wide block-ones lhsT (no partition_broadcast needed)
            ms = psM.tile([128, QB], F32, tag="ms", name=f"ms_{nm}")
            msB = psMB.tile([64, QB], F32, tag="msB", name=f"msB_{nm}")
            for half in range(QB // 512):
                hs = ts(half, 512)
                nc.tensor.matmul(ms[0:64, hs], lhsT=onesH0,
                                 rhs=sqA[:, hs], start=True, stop=True)
                nc.tensor.matmul(ms[64:128, hs], lhsT=onesH1,
                                 rhs=sqA[:, hs], start=True, stop=True)
                nc.tensor.matmul(msB[:, hs], lhsT=onesBB, rhs=sqB[:, hs],
                                 start=True, stop=True)
            mse = dnq.tile([128, QB], F32, tag="mse", name=f"mse_{nm}")
            nc.vector.tensor_scalar_add(mse, ms, 64.0e-6)
            lms = dnq.tile([128, QB], F32, tag="lms", name=f"lms_{nm}")
            nc.scalar.activation(lms, mse, AF.Ln, scale=1.0 / 64.0)
            ib = dnq.tile([128, QB], F32, tag="ib", name=f"ib_{nm}")
            nc.scalar.activation(ib, lms, AF.Exp, scale=-0.5)
            mseB = dnq.tile([64, QB], F32, tag="mseB", name=f"mseB_{nm}")
            nc.vector.tensor_scalar_add(mseB, msB, 64.0e-6)
            lmsB = dnq.tile([64, QB], F32, tag="lmsB", name=f"lmsB_{nm}")
            nc.scalar.activation(lmsB, mseB, AF.Ln, scale=1.0 / 64.0)
            ibB = dnq.tile([64, QB], F32, tag="ibB", name=f"ibB_{nm}")
            nc.scalar.activation(ibB, lmsB, AF.Exp, scale=-0.5)
            nc.vector.tensor_mul(d01[:, blks], yrA, ib)
            nc.vector.tensor_mul(d22[0:64, blks], yrB, ibB)
            nc.vector.tensor_mul(d22[64:128, blks], yrB, ibB)
            if probe and blk == 0 and d01 is qT01:
                nc.vector.tensor_copy(probe["pA"], pA)
                nc.vector.tensor_copy(probe["yrA"], yrA)
                nc.vector.tensor_copy(probe["sqA"], sqA)
                nc.vector.tensor_copy(probe["ms"], ms)
                nc.vector.tensor_copy(probe["ib"], ib)

        for blk in range(T // QB):
            qk_block(wqs, blk, qT01, qT22)
            qk_block(wks, blk, kT01, kT22)

    with tc.tile_pool(name="psV", bufs=2, space="PSUM") as psV:
        for t2 in range(NT // 2):
            v_ps = psV.tile([128, 2, 512], F32, tag="v_ps",
                            name=f"v_ps_{t2}")
            for j in range(2):
                t = 2 * t2 + j
                for ci in range(KC):
                    nc.tensor.matmul(v_ps[:, j, 0:NG],
                                     lhsT=xTs[:, ci, ts(t, 128)],
                                     rhs=wvs[:, ci, :],
                                     start=(ci == 0), stop=(ci == KC - 1))
            v_ps4 = v_ps[:, :, 0:NG].rearrange("p a (h d) -> p a h d", h=HL)
            nc.scalar.copy(v_all[:, 2 * t2:2 * t2 + 2, :, 0:64], v_ps4)

    # ===== pass 2: attention + projection, per tq chunk =====
    ppool = ctx.enter_context(tc.tile_pool(name="ppool", bufs=3))
    opool = ctx.enter_context(tc.tile_pool(name="opool", bufs=4))
    with tc.tile_pool(name="sps", bufs=1, space="PSUM") as sps, \
         tc.tile_pool(name="psY", bufs=1, space="PSUM") as psY:
        for tq in range(NTQ):
            tqs = ts(tq, TQB)
            yp = [psY.tile([65, TQB], F32, tag=f"yp{h}", bufs=1,
                           name=f"yp{h}_{tq}")
                  for h in range(HL)]
            for g in range(NT // 4):
                tkg = [4 * g + j for j in range(4)]
                # three 4-plane score tiles per 4-tk group, one fused exp
                # each; paired planes use opposite PE row halves.
                for tag, mm in (
                    ("sa", ((kT01, qT01, slice(0, 64), tkg[0], None, 0),
                            (kT01, qT01, slice(64, 128), tkg[0], (64, 0), 1),
                            (kT01, qT01, slice(0, 64), tkg[1], None, 0),
                            (kT01, qT01, slice(64, 128), tkg[1], (64, 0), 1))),
                    ("sb", ((kT01, qT01, slice(0, 64), tkg[2], None, 0),
                            (kT01, qT01, slice(64, 128), tkg[2], (64, 0), 1),
                            (kT01, qT01, slice(0, 64), tkg[3], None, 0),
                            (kT01, qT01, slice(64, 128), tkg[3], (64, 0), 1))),
                    ("sc", ((kT22, qT22, slice(0, 64), tkg[0], None, 2),
                            (kT22, qT22, slice(64, 128), tkg[1], (64, 0), 2),
                            (kT22, qT22, slice(0, 64), tkg[2], None, 2),
                            (kT22, qT22, slice(64, 128), tkg[3], (64, 0), 2))),
                ):
                    s = sps.tile([128, 4, TQB], F32, tag="s4",
                                 name=f"{tag}_{tq}_{g}")
                    for i, (kT, qT, half, tk, pos, _h) in enumerate(mm):
                        nc.tensor.matmul(s[:, i, :],
                                         lhsT=kT[half, ts(tk, 128)],
                                         rhs=qT[half, tqs],
                                         start=True, stop=True,
                                         tile_position=pos)
                    p = ppool.tile([128, 4, TQB], BF16, tag="p",
                                   name=f"p{tag}_{tq}_{g}")
                    nc.scalar.activation(p.rearrange("p a n -> p (a n)"),
                                         s.rearrange("p a n -> p (a n)"),
                                         AF.Exp, scale=0.125)
                    for i, (kT, qT, half, tk, pos, h) in enumerate(mm):
                        nc.tensor.matmul(yp[h], lhsT=v_all[:, tk, h, :],
                                         rhs=p[:, i, :],
                                         start=(tk == 0),
                                         stop=(tk == NT - 1))

            # normalize: row 64 of yp is the softmax denominator
            for h in range(HL):
                rec = dn.tile([1, TQB], F32, tag="rec", name=f"rec{h}_{tq}")
                nc.vector.reciprocal(rec, yp[h][64:65, :])
                rb = dn.tile([64, TQB], F32, tag="rb", name=f"rb{h}_{tq}")
                nc.gpsimd.partition_broadcast(rb, rec)
                if h == 0:
                    dst = yTa[0:64, tqs]
                elif h == 1:
                    dst = yTa[64:128, tqs]
                else:
                    dst = yTb[:, tqs]
                nc.vector.tensor_mul(dst, yp[h][0:64, :], rb)

            # projection for this tq chunk's 4 token tiles (bf16
            # partials); 2 tiles share one 4-plane psum tile (planes
            # padded to 512 f32 so each matmul dst is bank-aligned),
            # one wide copy per pair, one DMA per tq chunk.
            o_sb = opool.tile([128, 4, C], BF16, tag="o_sb",
                              name=f"o_sb_{tq}")
            for pr in range(2):
                pp = sps.tile([128, 4, 512], F32, tag="s4",
                              name=f"pp_{tq}_{pr}")
                for j in range(2):
                    t = 4 * tq + 2 * pr + j
                    for nh in range(2):
                        nc.tensor.matmul(pp[:, 2 * j + nh, 0:384],
                                         lhsT=yTa[:, ts(t, 128)],
                                         rhs=wpa_s[:, ts(nh, 384)],
                                         start=True, stop=False)
                        nc.tensor.matmul(pp[:, 2 * j + nh, 0:384],
                                         lhsT=yTb[:, ts(t, 128)],
                                         rhs=wpb_s[:, ts(nh, 384)],
                                         start=False, stop=True)
                dst = o_sb[:, 2 * pr:2 * pr + 2, :].rearrange(
                    "p a (b n) -> p a b n", b=2)
                srcv = pp[:, :, 0:384].rearrange(
                    "p (a b) n -> p a b n", b=2)
                nc.vector.tensor_copy(dst, srcv)
            nc.sync.dma_start(
                out=y[tqs, :].rearrange("(a p) n -> p a n", p=128),
                in_=o_sb)

    return (qT01, qT22, kT01, v_all, yTa, yTb, cosd_s, sind_s, probe)


def build_nc(reps=1):
    nc = bacc.Bacc("TRN2", target_bir_lowering=False, debug=False,
                   num_devices=8)
    xT = nc.dram_tensor("xT", [128, KC, T], BF16, kind="ExternalInput").ap()
    cosd = nc.dram_tensor("cosd", [32, T], F32, kind="ExternalInput").ap()
    sind = nc.dram_tensor("sind", [32, T], F32, kind="ExternalInput").ap()
    wq = nc.dram_tensor("wq", [128, KC, 3 * NG], BF16,
                        kind="ExternalInput").ap()
    wpa = nc.dram_tensor("wpa", [128, C], BF16, kind="ExternalInput").ap()
    wpb = nc.dram_tensor("wpb", [64, C], BF16, kind="ExternalInput").ap()
    y = nc.dram_tensor("y", [T, C], BF16, kind="ExternalOutput").ap()
    with tile.TileContext(nc) as tc:
        for _ in range(reps):
            with ExitStack() as ctx:
                build_kernel(tc, ctx, xT, cosd, sind, wq, wq, wq,
                             wpa, wpb, y)
    nc.compile()
    return nc


def make_in_maps(x, cos, sin, wq, wk, wv, wproj):
    x = np.asarray(x, np.float32)
    cosd = np.ascontiguousarray(
        np.asarray(cos, np.float32).reshape(T, 32).T)      # [32, T]
    sind = np.ascontiguousarray(
        np.asarray(sin, np.float32).reshape(T, 32).T)
    wq = np.asarray(wq, np.float32)
    wk = np.asarray(wk, np.float32)
    wv = np.asarray(wv, np.float32)
    wp = np.asarray(wproj, np.float32)

    def to_pcn(w):  # [768, n] f32 -> [128, 6, n] bf16
        n = w.shape[1]
        return np.ascontiguousarray(
            w.reshape(KC, 128, n).transpose(1, 0, 2)).astype(BF16NP)

    in_maps = []
    for cid in range(8):
        b, g = divmod(cid, 4)
        sl = slice(g * NG, (g + 1) * NG)
        xTb = np.ascontiguousarray(
            x[b].T.reshape(KC, 128, T).transpose(1, 0, 2)).astype(BF16NP)
        wf = np.concatenate([wq[:, sl], wk[:, sl], wv[:, sl]], axis=1)
        in_maps.append({
            "xT": xTb,
            "cosd": cosd,
            "sind": sind,
            "wq": to_pcn(wf),
            "wpa": np.ascontiguousarray(
                wp[g * NG:g * NG + 128, :]).astype(BF16NP),
            "wpb": np.ascontiguousarray(
                wp[g * NG + 128:(g + 1) * NG, :]).astype(BF16NP),
        })
    return in_maps


_NC = None


def kernel(x, cos, sin, wq, wk, wv, wproj):
    global _NC
    if _NC is None:
        _NC = build_nc()
    in_maps = make_in_maps(x, cos, sin, wq, wk, wv, wproj)
    res = run_bass_kernel_spmd(_NC, in_maps, list(range(8)))
    outs = [r["y"].astype(np.float32) for r in res.results]
    y0 = outs[0] + outs[1] + outs[2] + outs[3]
    y1 = outs[4] + outs[5] + outs[6] + outs[7]
    return np.stack([y0, y1], axis=0).astype(np.float32)


if __name__ == "__main__":
    rng = np.random.default_rng(0)
    ins = {
        "x": rng.standard_normal((2, T, C), dtype=np.float32),
        "cos": rng.random((T, 1, 32), dtype=np.float32),
        "sin": rng.random((T, 1, 32), dtype=np.float32),
        "wq": rng.standard_normal((C, C), dtype=np.float32) / np.sqrt(C),
        "wk": rng.standard_normal((C, C), dtype=np.float32) / np.sqrt(C),
        "wv": rng.standard_normal((C, C), dtype=np.float32) / np.sqrt(C),
        "wproj": rng.standard_normal((C, C), dtype=np.float32) / np.sqrt(C),
    }
    out = kernel(**ins)
    print(out.shape, out.dtype, np.abs(out).max())


# revision 21
# speedup vs baseline: 2.8946x; 1.0753x over previous
# Multi-head attention (B=2, T=2048, C=768, H=12, D=64) on 8 NeuronCores.
#
# Sharding: core i handles batch b = i // 4 and head group g = i % 4
# (3 heads each).  Host pre-transposes/casts inputs; each core computes
# q/k DIRECTLY in transposed [d, token] layout (lhsT = weight chunk,
# rhs = xT chunk), so no PE transposes are needed anywhere:
#   qT/kT[hd, tok] = sum_ci wq[ci*128:+128, hd]^T @ xT[ci, tok]
#   rope in transposed layout with host-duplicated cosT/sinT tables
#   rmsnorm via block-ones PE matmul (partition reduction) + Ln/Exp
#   scores s^T [tk,tq] = kT.T @ qT ; p = exp(s/8) (fused 2-head tiles)
#   AV: yplus += [v | 1].T @ p  (v computed in [tok, hd] layout)
#   softmax denom = ones-row of yplus; reciprocal + partition-broadcast
#   proj: out = yT.T @ wp slices -> bf16 partials, summed on host.

import numpy as np
from contextlib import ExitStack
import ml_dtypes

import concourse.hw_specs as _hw_specs
from concourse import mybir

AF = mybir.ActivationFunctionType
ALU = mybir.AluOpType

# Keep Exp/Ln in exactly one ACT table set so bacc's greedy set selection
# never bounces between table sets (each bounce is a ~1.3us table DMA).
if not getattr(_hw_specs, "_mha_act_patch", False):
    _orig_gat = _hw_specs.get_activation_tables

    def _gat_one_exp_ln_set(arch):
        tabs = _orig_gat(arch)
        for name, s in tabs.items():
            if name != "natural_log_exp_and_others":
                s.discard(AF.Exp)
                s.discard(AF.Ln)
        return tabs

    _hw_specs.get_activation_tables = _gat_one_exp_ln_set
    _hw_specs._mha_act_patch = True

import concourse.bass as bass          # noqa: E402
import concourse.tile as tile          # noqa: E402
from concourse import bacc             # noqa: E402
bacc.get_activation_tables = _hw_specs.get_activation_tables
from concourse.bass import ts          # noqa: E402
from concourse.bass_utils import run_bass_kernel_spmd  # noqa: E402

F32 = mybir.dt.float32
BF16 = mybir.dt.bfloat16
BF16NP = ml_dtypes.bfloat16

T = 2048
C = 768
HL = 3          # heads per core
D = 64
NG = HL * D     # 192, per-core qkv width
NT = T // 128   # 16 token tiles
KC = C // 128   # 6 contraction chunks
TQB = 512       # tq block
NTQ = T // TQB  # 4


def build_kernel(tc, ctx, xT, cosd, sind, wq, wk, wv, wpa, wpb, y):
    nc = tc.nc

    big = ctx.enter_context(tc.tile_pool(name="big", bufs=1))

    # ---- persistent inputs: one DMA each, already bf16/transposed ----
    xTs = big.tile([128, KC, T], BF16, tag="xTs")
    nc.sync.dma_start(out=xTs, in_=xT)
    ws = big.tile([128, KC, 3 * NG], BF16, tag="ws")
    nc.sync.dma_start(out=ws, in_=wq)   # wq dram tensor holds [wq|wk|wv]
    wqs = ws[:, :, 0:NG]
    wks = ws[:, :, NG:2 * NG]
    wvs = ws[:, :, 2 * NG:3 * NG]
    wpa_s = big.tile([128, C], BF16, tag="wpa_s")
    nc.sync.dma_start(out=wpa_s, in_=wpa)
    wpb_s = big.tile([64, C], BF16, tag="wpb_s")
    nc.sync.dma_start(out=wpb_s, in_=wpb)
    # cos/sin arrive as [32, T]; replicate to 4 row-blocks on device.
    # sin is stored SIGNED: rows j<32 = -sin (for the y2 = x2*c - x1*s
    # half after the 32-row swap), rows 32:64 = +sin; pattern repeats.
    cosd_s = big.tile([128, T], F32, tag="cosd_s")
    nc.sync.dma_start(out=cosd_s[0:32, :], in_=cosd)
    sind_s = big.tile([128, T], F32, tag="sind_s")
    nc.sync.dma_start(out=sind_s[32:64, :], in_=sind)
    nc.vector.tensor_scalar_mul(sind_s[0:32, :], sind_s[32:64, :], -1.0)
    for r in range(1, 4):
        nc.vector.tensor_copy(cosd_s[ts(r, 32), :], cosd_s[0:32, :])
    nc.vector.tensor_copy(sind_s[64:128, :], sind_s[0:64, :])

    # block-ones for the rmsnorm partition reduction; M=64-wide so the
    # reduction matmul also BROADCASTS the per-head sum to 64 rows
    onesH0 = big.tile([128, 64], BF16, tag="onesH0")
    nc.gpsimd.memset(onesH0, 0.0)
    nc.gpsimd.memset(onesH0[0:64, :], 1.0)
    onesH1 = big.tile([128, 64], BF16, tag="onesH1")
    nc.gpsimd.memset(onesH1, 0.0)
    nc.gpsimd.memset(onesH1[64:128, :], 1.0)
    onesBB = big.tile([64, 64], BF16, tag="onesBB")
    nc.gpsimd.memset(onesBB, 1.0)

    # ---- persistent big tensors ----
    qT01 = big.tile([128, T], BF16, tag="qT01")
    kT01 = big.tile([128, T], BF16, tag="kT01")
    qT22 = big.tile([128, T], BF16, tag="qT22")
    kT22 = big.tile([128, T], BF16, tag="kT22")
    yTa = big.tile([128, T], BF16, tag="yTa")   # rows 0:64 head0, 64:128 h1
    yTb = big.tile([64, T], BF16, tag="yTb")    # head2
    v_all = big.tile([128, NT, HL, 65], BF16, tag="v_all")
    nc.gpsimd.memset(v_all[:, :, :, 64:65], 1.0)

    work = ctx.enter_context(tc.tile_pool(name="work", bufs=1))
    dnq = ctx.enter_context(tc.tile_pool(name="dnq", bufs=1))
    dn = ctx.enter_context(tc.tile_pool(name="dn", bufs=2))

    # ===== pass 1: qT/kT via M-packed 128-row chunks + v tiles =====
    # The fused weight [wq|wk] columns are consumed in three 128-column
    # chunks: chunk0 = q heads 0,1; chunk1 = [q head2 | k head0];
    # chunk2 = k heads 1,2.  Each chunk's 128 psum rows are two 64-row
    # head blocks that share the whole rope/rmsnorm pipeline.
    QB = 1024   # qk processing block width (2 psum banks)
    with tc.tile_pool(name="psC", bufs=2, space="PSUM") as psC, \
         tc.tile_pool(name="psM", bufs=2, space="PSUM") as psM:

        # (tensor, row-slice or None, rows 0:64 dst, rows 64:128 dst)
        chunk_dsts = [
            ((qT01, slice(0, 128)),),                       # chunk 0
            ((qT22, slice(0, 64)), (qT22, slice(64, 128)),
             (kT01, slice(0, 64))),                         # chunk 1: see below
            ((kT01, slice(64, 128)), (kT22, slice(0, 64)),
             (kT22, slice(64, 128))),                       # chunk 2
        ]

        def qk_chunk(c3, blk):
            blks = ts(blk, QB)
            nm = f"c{c3}_{blk}"
            pC = psC.tile([128, QB], F32, tag="pC", name=f"pC_{nm}")
            for half in range(QB // 512):
                hs = ts(half, 512)
                bs = slice(blk * QB + half * 512,
                           blk * QB + half * 512 + 512)
                for ci in range(KC):
                    nc.tensor.matmul(pC[:, hs],
                                     lhsT=ws[:, ci, ts(c3, 128)],
                                     rhs=xTs[:, ci, bs],
                                     start=(ci == 0), stop=(ci == KC - 1))
            # rope: tc = x*cos; swapped sin-product written directly via
            # shifted-dst muls (srcs aligned, dst may shift); yr = tc+uSw
            tcC = work.tile([128, QB], F32, tag="tcC", name=f"tcC_{nm}")
            nc.vector.tensor_mul(tcC, pC, cosd_s[:, blks])
            uSw = work.tile([128, QB], F32, tag="uSw", name=f"uSw_{nm}")
            nc.vector.tensor_mul(uSw[0:32], pC[32:64], sind_s[32:64, blks])
            nc.vector.tensor_mul(uSw[32:64], pC[0:32], sind_s[0:32, blks])
            nc.vector.tensor_mul(uSw[64:96], pC[96:128],
                                 sind_s[96:128, blks])
            nc.vector.tensor_mul(uSw[96:128], pC[64:96],
                                 sind_s[64:96, blks])
            yr = work.tile([128, QB], F32, tag="yr", name=f"yr_{nm}")
            nc.vector.tensor_add(yr, tcC, uSw)
            # rmsnorm: block-ones matmul broadcasts each head's sum
            sq = work.tile([128, QB], BF16, tag="sq", name=f"sq_{nm}")
            nc.vector.tensor_mul(sq, yr, yr)
            ms = psM.tile([128, QB], F32, tag="ms", name=f"ms_{nm}")
            for half in range(QB // 512):
                hs = ts(half, 512)
                nc.tensor.matmul(ms[0:64, hs], lhsT=onesH0,
                                 rhs=sq[:, hs], start=True, stop=True)
                nc.tensor.matmul(ms[64:128, hs], lhsT=onesH1,
                                 rhs=sq[:, hs], start=True, stop=True)
            mse = dnq.tile([128, QB], F32, tag="mse", name=f"mse_{nm}")
            nc.vector.tensor_scalar_add(mse, ms, 64.0e-6)
            lms = dnq.tile([128, QB], F32, tag="lms", name=f"lms_{nm}")
            nc.scalar.activation(lms, mse, AF.Ln, scale=1.0 / 64.0)
            ib = dnq.tile([128, QB], F32, tag="ib", name=f"ib_{nm}")
            nc.scalar.activation(ib, lms, AF.Exp, scale=-0.5)
            # scaled bf16 writes to the packed score tensors
            if c3 == 0:
                nc.vector.tensor_mul(qT01[:, blks], yr, ib)
            elif c3 == 1:
                nc.vector.tensor_mul(qT22[0:64, blks], yr[0:64], ib[0:64])
                nc.vector.tensor_mul(qT22[64:128, blks], yr[0:64], ib[0:64])
                nc.vector.tensor_mul(kT01[0:64, blks], yr[64:128],
                                     ib[64:128])
            else:
                nc.vector.tensor_mul(kT01[64:128, blks], yr[0:64],
                                     ib[0:64])
                nc.vector.tensor_mul(kT22[0:64, blks], yr[64:128],
                                     ib[64:128])
                nc.vector.tensor_mul(kT22[64:128, blks], yr[64:128],
                                     ib[64:128])

        for blk in range(T // QB):
            for c3 in range(3):
                qk_chunk(c3, blk)

    with tc.tile_pool(name="psV", bufs=2, space="PSUM") as psV:
        for t2 in range(NT // 2):
            v_ps = psV.tile([128, 2, 512], F32, tag="v_ps",
                            name=f"v_ps_{t2}")
            for j in range(2):
                t = 2 * t2 + j
                for ci in range(KC):
                    nc.tensor.matmul(v_ps[:, j, 0:NG],
                                     lhsT=xTs[:, ci, ts(t, 128)],
                                     rhs=wvs[:, ci, :],
                                     start=(ci == 0), stop=(ci == KC - 1))
            v_ps4 = v_ps[:, :, 0:NG].rearrange("p a (h d) -> p a h d", h=HL)
            nc.scalar.copy(v_all[:, 2 * t2:2 * t2 + 2, :, 0:64], v_ps4)

    # ===== pass 2: attention + projection, per tq chunk =====
    ppool = ctx.enter_context(tc.tile_pool(name="ppool", bufs=3))
    opool = ctx.enter_context(tc.tile_pool(name="opool", bufs=4))
    with tc.tile_pool(name="sps", bufs=1, space="PSUM") as sps, \
         tc.tile_pool(name="psY", bufs=1, space="PSUM") as psY:
        for tq in range(NTQ):
            tqs = ts(tq, TQB)
            yp = [psY.tile([65, TQB], F32, tag=f"yp{h}", bufs=1,
                           name=f"yp{h}_{tq}")
                  for h in range(HL)]
            for g in range(NT // 4):
                tkg = [4 * g + j for j in range(4)]
                # three 4-plane score tiles per 4-tk group, one fused exp
                # each; paired planes use opposite PE row halves.
                for tag, mm in (
                    ("sa", ((kT01, qT01, slice(0, 64), tkg[0], None, 0),
                            (kT01, qT01, slice(64, 128), tkg[0], (64, 0), 1),
                            (kT01, qT01, slice(0, 64), tkg[1], None, 0),
                            (kT01, qT01, slice(64, 128), tkg[1], (64, 0), 1))),
                    ("sb", ((kT01, qT01, slice(0, 64), tkg[2], None, 0),
                            (kT01, qT01, slice(64, 128), tkg[2], (64, 0), 1),
                            (kT01, qT01, slice(0, 64), tkg[3], None, 0),
                            (kT01, qT01, slice(64, 128), tkg[3], (64, 0), 1))),
                    ("sc", ((kT22, qT22, slice(0, 64), tkg[0], None, 2),
                            (kT22, qT22, slice(64, 128), tkg[1], (64, 0), 2),
                            (kT22, qT22, slice(0, 64), tkg[2], None, 2),
                            (kT22, qT22, slice(64, 128), tkg[3], (64, 0), 2))),
                ):
                    s = sps.tile([128, 4, TQB], F32, tag="s4",
                                 name=f"{tag}_{tq}_{g}")
                    for i, (kT, qT, half, tk, pos, _h) in enumerate(mm):
                        nc.tensor.matmul(s[:, i, :],
                                         lhsT=kT[half, ts(tk, 128)],
                                         rhs=qT[half, tqs],
                                         start=True, stop=True,
                                         tile_position=pos)
                    p = ppool.tile([128, 4, TQB], BF16, tag="p",
                                   name=f"p{tag}_{tq}_{g}")
                    nc.scalar.activation(p.rearrange("p a n -> p (a n)"),
                                         s.rearrange("p a n -> p (a n)"),
                                         AF.Exp, scale=0.125)
                    for i, (kT, qT, half, tk, pos, h) in enumerate(mm):
                        nc.tensor.matmul(yp[h], lhsT=v_all[:, tk, h, :],
                                         rhs=p[:, i, :],
                                         start=(tk == 0),
                                         stop=(tk == NT - 1))

            # normalize: row 64 of yp is the softmax denominator
            for h in range(HL):
                rec = dn.tile([1, TQB], F32, tag="rec", name=f"rec{h}_{tq}")
                nc.vector.reciprocal(rec, yp[h][64:65, :])
                rb = dn.tile([64, TQB], F32, tag="rb", name=f"rb{h}_{tq}")
                nc.gpsimd.partition_broadcast(rb, rec)
                if h == 0:
                    dst = yTa[0:64, tqs]
                elif h == 1:
                    dst = yTa[64:128, tqs]
                else:
                    dst = yTb[:, tqs]
                nc.vector.tensor_mul(dst, yp[h][0:64, :], rb)

            # projection for this tq chunk's 4 token tiles (bf16
            # partials); 2 tiles share one 4-plane psum tile (planes
            # padded to 512 f32 so each matmul dst is bank-aligned),
            # one wide copy per pair, one DMA per tq chunk.
            o_sb = opool.tile([128, 4, C], BF16, tag="o_sb",
                              name=f"o_sb_{tq}")
            for pr in range(2):
                pp = sps.tile([128, 4, 512], F32, tag="s4",
                              name=f"pp_{tq}_{pr}")
                for j in range(2):
                    t = 4 * tq + 2 * pr + j
                    for nh in range(2):
                        nc.tensor.matmul(pp[:, 2 * j + nh, 0:384],
                                         lhsT=yTa[:, ts(t, 128)],
                                         rhs=wpa_s[:, ts(nh, 384)],
                                         start=True, stop=False)
                        nc.tensor.matmul(pp[:, 2 * j + nh, 0:384],
                                         lhsT=yTb[:, ts(t, 128)],
                                         rhs=wpb_s[:, ts(nh, 384)],
                                         start=False, stop=True)
                dst = o_sb[:, 2 * pr:2 * pr + 2, :].rearrange(
                    "p a (b n) -> p a b n", b=2)
                srcv = pp[:, :, 0:384].rearrange(
                    "p (a b) n -> p a b n", b=2)
                nc.vector.tensor_copy(dst, srcv)
            nc.sync.dma_start(
                out=y[tqs, :].rearrange("(a p) n -> p a n", p=128),
                in_=o_sb)

    return (qT01, qT22, kT01, v_all, yTa, yTb, cosd_s, sind_s)


def build_nc(reps=1):
    nc = bacc.Bacc("TRN2", target_bir_lowering=False, debug=False,
                   num_devices=8)
    xT = nc.dram_tensor("xT", [128, KC, T], BF16, kind="ExternalInput").ap()
    cosd = nc.dram_tensor("cosd", [32, T], F32, kind="ExternalInput").ap()
    sind = nc.dram_tensor("sind", [32, T], F32, kind="ExternalInput").ap()
    wq = nc.dram_tensor("wq", [128, KC, 3 * NG], BF16,
                        kind="ExternalInput").ap()
    wpa = nc.dram_tensor("wpa", [128, C], BF16, kind="ExternalInput").ap()
    wpb = nc.dram_tensor("wpb", [64, C], BF16, kind="ExternalInput").ap()
    y = nc.dram_tensor("y", [T, C], BF16, kind="ExternalOutput").ap()
    with tile.TileContext(nc) as tc:
        for _ in range(reps):
            with ExitStack() as ctx:
                build_kernel(tc, ctx, xT, cosd, sind, wq, wq, wq,
                             wpa, wpb, y)
    nc.compile()
    return nc


def make_in_maps(x, cos, sin, wq, wk, wv, wproj):
    x = np.asarray(x, np.float32)
    cosd = np.ascontiguousarray(
        np.asarray(cos, np.float32).reshape(T, 32).T)      # [32, T]
    sind = np.ascontiguousarray(
        np.asarray(sin, np.float32).reshape(T, 32).T)
    wq = np.asarray(wq, np.float32)
    wk = np.asarray(wk, np.float32)
    wv = np.asarray(wv, np.float32)
    wp = np.asarray(wproj, np.float32)

    def to_pcn(w):  # [768, n] f32 -> [128, 6, n] bf16
        n = w.shape[1]
        return np.ascontiguousarray(
            w.reshape(KC, 128, n).transpose(1, 0, 2)).astype(BF16NP)

    in_maps = []
    for cid in range(8):
        b, g = divmod(cid, 4)
        sl = slice(g * NG, (g + 1) * NG)
        xTb = np.ascontiguousarray(
            x[b].T.reshape(KC, 128, T).transpose(1, 0, 2)).astype(BF16NP)
        wf = np.concatenate([wq[:, sl], wk[:, sl], wv[:, sl]], axis=1)
        in_maps.append({
            "xT": xTb,
            "cosd": cosd,
            "sind": sind,
            "wq": to_pcn(wf),
            "wpa": np.ascontiguousarray(
                wp[g * NG:g * NG + 128, :]).astype(BF16NP),
            "wpb": np.ascontiguousarray(
                wp[g * NG + 128:(g + 1) * NG, :]).astype(BF16NP),
        })
    return in_maps


_NC = None


def kernel(x, cos, sin, wq, wk, wv, wproj):
    global _NC
    if _NC is None:
        _NC = build_nc()
    in_maps = make_in_maps(x, cos, sin, wq, wk, wv, wproj)
    res = run_bass_kernel_spmd(_NC, in_maps, list(range(8)))
    outs = [r["y"].astype(np.float32) for r in res.results]
    y0 = outs[0] + outs[1] + outs[2] + outs[3]
    y1 = outs[4] + outs[5] + outs[6] + outs[7]
    return np.stack([y0, y1], axis=0).astype(np.float32)


if __name__ == "__main__":
    rng = np.random.default_rng(0)
    ins = {
        "x": rng.standard_normal((2, T, C), dtype=np.float32),
        "cos": rng.random((T, 1, 32), dtype=np.float32),
        "sin": rng.random((T, 1, 32), dtype=np.float32),
        "wq": rng.standard_normal((C, C), dtype=np.float32) / np.sqrt(C),
        "wk": rng.standard_normal((C, C), dtype=np.float32) / np.sqrt(C),
        "wv": rng.standard_normal((C, C), dtype=np.float32) / np.sqrt(C),
        "wproj": rng.standard_normal((C, C), dtype=np.float32) / np.sqrt(C),
    }
    out = kernel(**ins)
    print(out.shape, out.dtype, np.abs(out).max())


# revision 22
# speedup vs baseline: 3.4332x; 1.1861x over previous
# Multi-head attention (B=2, T=2048, C=768, H=12, D=64) on 8 NeuronCores.
#
# Sharding: core i handles batch b = i // 4 and head group g = i % 4
# (3 heads each).  Host pre-transposes/casts inputs; each core computes
# q/k DIRECTLY in transposed [d, token] layout (lhsT = weight chunk,
# rhs = xT chunk), so no PE transposes are needed anywhere:
#   qT/kT[hd, tok] = sum_ci wq[ci*128:+128, hd]^T @ xT[ci, tok]
#   rope in transposed layout with host-duplicated cosT/sinT tables
#   rmsnorm via block-ones PE matmul (partition reduction) + Ln/Exp
#   scores s^T [tk,tq] = kT.T @ qT ; p = exp(s/8) (fused 2-head tiles)
#   AV: yplus += [v | 1].T @ p  (v computed in [tok, hd] layout)
#   softmax denom = ones-row of yplus; reciprocal + partition-broadcast
#   proj: out = yT.T @ wp slices -> bf16 partials, summed on host.

import numpy as np
from contextlib import ExitStack
import ml_dtypes

import concourse.hw_specs as _hw_specs
from concourse import mybir

AF = mybir.ActivationFunctionType
ALU = mybir.AluOpType

# Keep Exp/Ln in exactly one ACT table set so bacc's greedy set selection
# never bounces between table sets (each bounce is a ~1.3us table DMA).
if not getattr(_hw_specs, "_mha_act_patch", False):
    _orig_gat = _hw_specs.get_activation_tables

    def _gat_one_exp_ln_set(arch):
        tabs = _orig_gat(arch)
        for name, s in tabs.items():
            if name != "natural_log_exp_and_others":
                s.discard(AF.Exp)
                s.discard(AF.Ln)
        return tabs

    _hw_specs.get_activation_tables = _gat_one_exp_ln_set
    _hw_specs._mha_act_patch = True

import concourse.bass as bass          # noqa: E402
import concourse.tile as tile          # noqa: E402
from concourse import bacc             # noqa: E402
bacc.get_activation_tables = _hw_specs.get_activation_tables
from concourse.bass import ts          # noqa: E402
from concourse.bass_utils import run_bass_kernel_spmd  # noqa: E402

F32 = mybir.dt.float32
BF16 = mybir.dt.bfloat16
BF16NP = ml_dtypes.bfloat16

T = 2048
C = 768
HL = 3          # heads per core
D = 64
NG = HL * D     # 192, per-core qkv width
NT = T // 128   # 16 token tiles
KC = C // 128   # 6 contraction chunks
TQB = 512       # tq block
NTQ = T // TQB  # 4


def build_kernel(tc, ctx, xT, cosd, sind, wq, wk, wv, wpa, wpb, y):
    nc = tc.nc

    big = ctx.enter_context(tc.tile_pool(name="big", bufs=1))

    # ---- persistent inputs: one DMA each, already bf16/transposed ----
    xTs = big.tile([128, KC, T], BF16, tag="xTs")
    nc.sync.dma_start(out=xTs, in_=xT)
    ws = big.tile([128, KC, 3 * NG], BF16, tag="ws")
    nc.sync.dma_start(out=ws, in_=wq)   # wq dram tensor holds [wq|wk|wv]
    wqs = ws[:, :, 0:NG]
    wks = ws[:, :, NG:2 * NG]
    wvs = ws[:, :, 2 * NG:3 * NG]
    wpa_s = big.tile([128, C], BF16, tag="wpa_s")
    nc.sync.dma_start(out=wpa_s, in_=wpa)
    wpb_s = big.tile([64, C], BF16, tag="wpb_s")
    nc.sync.dma_start(out=wpb_s, in_=wpb)
    # cos/sin arrive as [32, T]; replicate to 4 row-blocks on device.
    # sin is stored SIGNED: rows j<32 = -sin (for the y2 = x2*c - x1*s
    # half after the 32-row swap), rows 32:64 = +sin; pattern repeats.
    cosd_s = big.tile([128, T], F32, tag="cosd_s")
    nc.sync.dma_start(out=cosd_s[0:32, :], in_=cosd)
    sind_s = big.tile([128, T], F32, tag="sind_s")
    nc.sync.dma_start(out=sind_s[32:64, :], in_=sind)
    nc.vector.tensor_scalar_mul(sind_s[0:32, :], sind_s[32:64, :], -1.0)
    for r in range(1, 4):
        nc.vector.tensor_copy(cosd_s[ts(r, 32), :], cosd_s[0:32, :])
    nc.vector.tensor_copy(sind_s[64:128, :], sind_s[0:64, :])

    # block-ones for the rmsnorm partition reduction; M=64-wide so the
    # reduction matmul also BROADCASTS the per-head sum to 64 rows
    onesH0 = big.tile([128, 64], BF16, tag="onesH0")
    nc.gpsimd.memset(onesH0, 0.0)
    nc.gpsimd.memset(onesH0[0:64, :], 1.0)
    onesH1 = big.tile([128, 64], BF16, tag="onesH1")
    nc.gpsimd.memset(onesH1, 0.0)
    nc.gpsimd.memset(onesH1[64:128, :], 1.0)
    onesBB = big.tile([64, 64], BF16, tag="onesBB")
    nc.gpsimd.memset(onesBB, 1.0)

    # ---- persistent big tensors ----
    qT01 = big.tile([128, T], BF16, tag="qT01")
    kT01 = big.tile([128, T], BF16, tag="kT01")
    qT22 = big.tile([128, T], BF16, tag="qT22")
    kT22 = big.tile([128, T], BF16, tag="kT22")
    yTa = big.tile([128, T], BF16, tag="yTa")   # rows 0:64 head0, 64:128 h1
    yTb = big.tile([64, T], BF16, tag="yTb")    # head2
    v_all = big.tile([128, NT, HL, 65], BF16, tag="v_all")
    nc.gpsimd.memset(v_all[:, :, :, 64:65], 1.0)

    work = ctx.enter_context(tc.tile_pool(name="work", bufs=1))
    dnq = ctx.enter_context(tc.tile_pool(name="dnq", bufs=1))
    dn = ctx.enter_context(tc.tile_pool(name="dn", bufs=2))

    # ===== pass 1: qT/kT via M-packed 128-row chunks + v tiles =====
    # The fused weight [wq|wk] columns are consumed in three 128-column
    # chunks: chunk0 = q heads 0,1; chunk1 = [q head2 | k head0];
    # chunk2 = k heads 1,2.  Each chunk's 128 psum rows are two 64-row
    # head blocks that share the whole rope/rmsnorm pipeline.
    QB = 1024   # qk processing block width (2 psum banks)
    with tc.tile_pool(name="psC", bufs=2, space="PSUM") as psC, \
         tc.tile_pool(name="psM", bufs=2, space="PSUM") as psM:

        def qk_chunk(c3, blk):
            blks = ts(blk, QB)
            nm = f"c{c3}_{blk}"
            pC = psC.tile([128, QB], F32, tag="pC", name=f"pC_{nm}")
            for half in range(QB // 512):
                hs = ts(half, 512)
                bs = slice(blk * QB + half * 512,
                           blk * QB + half * 512 + 512)
                for ci in range(KC):
                    nc.tensor.matmul(pC[:, hs],
                                     lhsT=ws[:, ci, ts(c3, 128)],
                                     rhs=xTs[:, ci, bs],
                                     start=(ci == 0), stop=(ci == KC - 1))
            # rope: tc = x*cos; swapped sin-product written directly via
            # shifted-dst muls (srcs aligned, dst may shift); yr = tc+uSw
            tcC = work.tile([128, QB], F32, tag="tcC", name=f"tcC_{nm}")
            nc.vector.tensor_mul(tcC, pC, cosd_s[:, blks])
            uSw = work.tile([128, QB], F32, tag="uSw", name=f"uSw_{nm}")
            nc.vector.tensor_mul(uSw[0:32], pC[32:64], sind_s[32:64, blks])
            nc.vector.tensor_mul(uSw[32:64], pC[0:32], sind_s[0:32, blks])
            nc.vector.tensor_mul(uSw[64:96], pC[96:128],
                                 sind_s[96:128, blks])
            nc.vector.tensor_mul(uSw[96:128], pC[64:96],
                                 sind_s[64:96, blks])
            yr = work.tile([128, QB], F32, tag="yr", name=f"yr_{nm}")
            nc.vector.tensor_add(yr, tcC, uSw)
            # rmsnorm: block-ones matmul broadcasts each head's sum
            sq = work.tile([128, QB], BF16, tag="sq", name=f"sq_{nm}")
            nc.vector.tensor_mul(sq, yr, yr)
            ms = psM.tile([128, QB], F32, tag="ms", name=f"ms_{nm}")
            for half in range(QB // 512):
                hs = ts(half, 512)
                nc.tensor.matmul(ms[0:64, hs], lhsT=onesH0,
                                 rhs=sq[:, hs], start=True, stop=True)
                nc.tensor.matmul(ms[64:128, hs], lhsT=onesH1,
                                 rhs=sq[:, hs], start=True, stop=True)
            mse = dnq.tile([128, QB], F32, tag="mse", name=f"mse_{nm}")
            nc.vector.tensor_scalar_add(mse, ms, 64.0e-6)
            lms = dnq.tile([128, QB], F32, tag="lms", name=f"lms_{nm}")
            nc.scalar.activation(lms, mse, AF.Ln, scale=1.0 / 64.0)
            ib = dnq.tile([128, QB], F32, tag="ib", name=f"ib_{nm}")
            nc.scalar.activation(ib, lms, AF.Exp, scale=-0.5)
            # scaled bf16 writes to the packed score tensors
            if c3 == 0:
                nc.vector.tensor_mul(qT01[:, blks], yr, ib)
            elif c3 == 1:
                nc.vector.tensor_mul(qT22[0:64, blks], yr[0:64], ib[0:64])
                nc.vector.tensor_mul(qT22[64:128, blks], yr[0:64], ib[0:64])
                nc.vector.tensor_mul(kT01[0:64, blks], yr[64:128],
                                     ib[64:128])
            else:
                nc.vector.tensor_mul(kT01[64:128, blks], yr[0:64],
                                     ib[0:64])
                nc.vector.tensor_mul(kT22[0:64, blks], yr[64:128],
                                     ib[64:128])
                nc.vector.tensor_mul(kT22[64:128, blks], yr[64:128],
                                     ib[64:128])

        for blk in range(T // QB):
            for c3 in range(3):
                qk_chunk(c3, blk)

    with tc.tile_pool(name="psV", bufs=2, space="PSUM") as psV:
        for t2 in range(NT // 2):
            v_ps = psV.tile([128, 2, 512], F32, tag="v_ps",
                            name=f"v_ps_{t2}")
            for j in range(2):
                t = 2 * t2 + j
                for ci in range(KC):
                    nc.tensor.matmul(v_ps[:, j, 0:NG],
                                     lhsT=xTs[:, ci, ts(t, 128)],
                                     rhs=wvs[:, ci, :],
                                     start=(ci == 0), stop=(ci == KC - 1))
            v_ps4 = v_ps[:, :, 0:NG].rearrange("p a (h d) -> p a h d", h=HL)
            nc.scalar.copy(v_all[:, 2 * t2:2 * t2 + 2, :, 0:64], v_ps4)

    # ===== pass 2: attention + projection, per tq chunk =====
    ppool = ctx.enter_context(tc.tile_pool(name="ppool", bufs=3))
    opool = ctx.enter_context(tc.tile_pool(name="opool", bufs=4))
    with tc.tile_pool(name="sps", bufs=1, space="PSUM") as sps, \
         tc.tile_pool(name="psY", bufs=1, space="PSUM") as psY:
        for tq in range(NTQ):
            tqs = ts(tq, TQB)
            yp = [psY.tile([65, TQB], F32, tag=f"yp{h}", bufs=1,
                           name=f"yp{h}_{tq}")
                  for h in range(HL)]
            for g in range(NT // 4):
                tkg = [4 * g + j for j in range(4)]
                # three 4-plane score tiles per 4-tk group, one fused exp
                # each; paired planes use opposite PE row halves.
                for tag, mm in (
                    ("sa", ((kT01, qT01, slice(0, 64), tkg[0], None, 0),
                            (kT01, qT01, slice(64, 128), tkg[0], (64, 0), 1),
                            (kT01, qT01, slice(0, 64), tkg[1], None, 0),
                            (kT01, qT01, slice(64, 128), tkg[1], (64, 0), 1))),
                    ("sb", ((kT01, qT01, slice(0, 64), tkg[2], None, 0),
                            (kT01, qT01, slice(64, 128), tkg[2], (64, 0), 1),
                            (kT01, qT01, slice(0, 64), tkg[3], None, 0),
                            (kT01, qT01, slice(64, 128), tkg[3], (64, 0), 1))),
                    ("sc", ((kT22, qT22, slice(0, 64), tkg[0], None, 2),
                            (kT22, qT22, slice(64, 128), tkg[1], (64, 0), 2),
                            (kT22, qT22, slice(0, 64), tkg[2], None, 2),
                            (kT22, qT22, slice(64, 128), tkg[3], (64, 0), 2))),
                ):
                    s = sps.tile([128, 4, TQB], F32, tag="s4",
                                 name=f"{tag}_{tq}_{g}")
                    for i, (kT, qT, half, tk, pos, _h) in enumerate(mm):
                        nc.tensor.matmul(s[:, i, :],
                                         lhsT=kT[half, ts(tk, 128)],
                                         rhs=qT[half, tqs],
                                         start=True, stop=True,
                                         tile_position=pos)
                    p = ppool.tile([128, 4, TQB], BF16, tag="p",
                                   name=f"p{tag}_{tq}_{g}")
                    nc.scalar.activation(p.rearrange("p a n -> p (a n)"),
                                         s.rearrange("p a n -> p (a n)"),
                                         AF.Exp, scale=0.125)
                    for i, (kT, qT, half, tk, pos, h) in enumerate(mm):
                        nc.tensor.matmul(yp[h], lhsT=v_all[:, tk, h, :],
                                         rhs=p[:, i, :],
                                         start=(tk == 0),
                                         stop=(tk == NT - 1))

            # normalize: row 64 of yp is the softmax denominator
            for h in range(HL):
                rec = dn.tile([1, TQB], F32, tag="rec", name=f"rec{h}_{tq}")
                nc.vector.reciprocal(rec, yp[h][64:65, :])
                rb = dn.tile([64, TQB], F32, tag="rb", name=f"rb{h}_{tq}")
                nc.gpsimd.partition_broadcast(rb, rec)
                if h == 0:
                    dst = yTa[0:64, tqs]
                elif h == 1:
                    dst = yTa[64:128, tqs]
                else:
                    dst = yTb[:, tqs]
                nc.vector.tensor_mul(dst, yp[h][0:64, :], rb)

            # projection for this tq chunk's 4 token tiles (bf16
            # partials); 2 tiles share one 4-plane psum tile (planes
            # padded to 512 f32 so each matmul dst is bank-aligned),
            # one wide copy per pair, one DMA per tq chunk.
            o_sb = opool.tile([128, 4, C], BF16, tag="o_sb",
                              name=f"o_sb_{tq}")
            for pr in range(2):
                pp = sps.tile([128, 4, 512], F32, tag="s4",
                              name=f"pp_{tq}_{pr}")
                for j in range(2):
                    t = 4 * tq + 2 * pr + j
                    for nh in range(2):
                        nc.tensor.matmul(pp[:, 2 * j + nh, 0:384],
                                         lhsT=yTa[:, ts(t, 128)],
                                         rhs=wpa_s[:, ts(nh, 384)],
                                         start=True, stop=False)
                        nc.tensor.matmul(pp[:, 2 * j + nh, 0:384],
                                         lhsT=yTb[:, ts(t, 128)],
                                         rhs=wpb_s[:, ts(nh, 384)],
                                         start=False, stop=True)
                dst = o_sb[:, 2 * pr:2 * pr + 2, :].rearrange(
                    "p a (b n) -> p a b n", b=2)
                srcv = pp[:, :, 0:384].rearrange(
                    "p (a b) n -> p a b n", b=2)
                nc.vector.tensor_copy(dst, srcv)
            nc.sync.dma_start(
                out=y[tqs, :].rearrange("(a p) n -> p a n", p=128),
                in_=o_sb)

    return (qT01, qT22, kT01, v_all, yTa, yTb, cosd_s, sind_s)


def build_nc(reps=1):
    nc = bacc.Bacc("TRN2", target_bir_lowering=False, debug=False,
                   num_devices=8)
    xT = nc.dram_tensor("xT", [128, KC, T], BF16, kind="ExternalInput").ap()
    cosd = nc.dram_tensor("cosd", [32, T], F32, kind="ExternalInput").ap()
    sind = nc.dram_tensor("sind", [32, T], F32, kind="ExternalInput").ap()
    wq = nc.dram_tensor("wq", [128, KC, 3 * NG], BF16,
                        kind="ExternalInput").ap()
    wpa = nc.dram_tensor("wpa", [128, C], BF16, kind="ExternalInput").ap()
    wpb = nc.dram_tensor("wpb", [64, C], BF16, kind="ExternalInput").ap()
    y = nc.dram_tensor("y", [T, C], BF16, kind="ExternalOutput").ap()
    with tile.TileContext(nc) as tc:
        for _ in range(reps):
            with ExitStack() as ctx:
                build_kernel(tc, ctx, xT, cosd, sind, wq, wq, wq,
                             wpa, wpb, y)
    nc.compile()
    return nc


def make_in_maps(x, cos, sin, wq, wk, wv, wproj):
    x = np.asarray(x, np.float32)
    cosd = np.ascontiguousarray(
        np.asarray(cos, np.float32).reshape(T, 32).T)      # [32, T]
    sind = np.ascontiguousarray(
        np.asarray(sin, np.float32).reshape(T, 32).T)
    wq = np.asarray(wq, np.float32)
    wk = np.asarray(wk, np.float32)
    wv = np.asarray(wv, np.float32)
    wp = np.asarray(wproj, np.float32)

    def to_pcn(w):  # [768, n] f32 -> [128, 6, n] bf16
        n = w.shape[1]
        return np.ascontiguousarray(
            w.reshape(KC, 128, n).transpose(1, 0, 2)).astype(BF16NP)

    in_maps = []
    for cid in range(8):
        b, g = divmod(cid, 4)
        sl = slice(g * NG, (g + 1) * NG)
        xTb = np.ascontiguousarray(
            x[b].T.reshape(KC, 128, T).transpose(1, 0, 2)).astype(BF16NP)
        wf = np.concatenate([wq[:, sl], wk[:, sl], wv[:, sl]], axis=1)
        in_maps.append({
            "xT": xTb,
            "cosd": cosd,
            "sind": sind,
            "wq": to_pcn(wf),
            "wpa": np.ascontiguousarray(
                wp[g * NG:g * NG + 128, :]).astype(BF16NP),
            "wpb": np.ascontiguousarray(
                wp[g * NG + 128:(g + 1) * NG, :]).astype(BF16NP),
        })
    return in_maps


_NC = None


def kernel(x, cos, sin, wq, wk, wv, wproj):
    global _NC
    if _NC is None:
        _NC = build_nc()
    in_maps = make_in_maps(x, cos, sin, wq, wk, wv, wproj)
    res = run_bass_kernel_spmd(_NC, in_maps, list(range(8)))
    outs = [r["y"].astype(np.float32) for r in res.results]
    y0 = outs[0] + outs[1] + outs[2] + outs[3]
    y1 = outs[4] + outs[5] + outs[6] + outs[7]
    return np.stack([y0, y1], axis=0).astype(np.float32)


if __name__ == "__main__":
    rng = np.random.default_rng(0)
    ins = {
        "x": rng.standard_normal((2, T, C), dtype=np.float32),
        "cos": rng.random((T, 1, 32), dtype=np.float32),
        "sin": rng.random((T, 1, 32), dtype=np.float32),
        "wq": rng.standard_normal((C, C), dtype=np.float32) / np.sqrt(C),
        "wk": rng.standard_normal((C, C), dtype=np.float32) / np.sqrt(C),
        "wv": rng.standard_normal((C, C), dtype=np.float32) / np.sqrt(C),
        "wproj": rng.standard_normal((C, C), dtype=np.float32) / np.sqrt(C),
    }
    out = kernel(**ins)
    print(out.shape, out.dtype, np.abs(out).max())
